# revision 1
# baseline (speedup 1.0000x reference)
"""Linear attention (silu+1 feature map) MultiHeadAttention kernel for 8x TRN2.

Sharding: data-parallel over batch (B=8 -> 1 batch element per NeuronCore).
Per-core math (T=4096, D=1024, H=16, Dh=64), all matmuls bf16 / fp32 PSUM:

  phase 1 (stream token tiles):
    qT[o,t]   = WqT.T @ xT          (feature-major, stationary = WqT chunks)
    phi_qT    = silu(s*qT + s*bq) + 1        (stored bf16, feature-major)
    k[t,e]    = xT.T @ WkT          (token-major, stationary = xT chunks)
    v[t,e]    = xT.T @ WvT + bv
    phi_k     = silu(s*k) + 1
    vk_h[e,d] += v_h.T @ phi_k_h    (PSUM accumulate, 16 heads packed in 1 bank)
  M stage:
    M_h[d,o]  = vk_h.T @ WoT_h      ->  M = vstack_h(M_h)   [1024,1024] bf16
  phase 2:
    yT[o,t]   = M.T @ phi_qT + bo   (one dense GEMM; folds per-head phi_q@kv
                                     and the output projection together)

Host side: transposes x per batch, pre-transposes/casts weights to bf16,
gathers yT.T per core. Output fp32.
"""

import numpy as np
import ml_dtypes

B, T, D = 8, 4096, 1024
H, DH = 16, 64
SCALE = float(DH ** -0.25)
NCORES = 8
P = 128
DC = D // P          # 8 feature chunks
TT = 512             # token tile (phase 1)
NTT = T // TT        # 8 token tiles
NSUB = TT // P       # 4 sub-tiles of 128 tokens

_BF16 = ml_dtypes.bfloat16

_CACHE = {}


def _split_multi_waits(nc):
    """walrus in this container only encodes ONE sync-wait command per
    instruction. Hoist extra waits onto injected same-engine NOPs placed
    immediately before the instruction (program order on the engine queue
    makes this semantically identical)."""
    import concourse.mybir as mybir

    n_split = 0
    for fn in nc.m.functions:
        for bb in fn.blocks:
            new = []
            changed = False
            for inst in bb.instructions:
                si = inst.sync_info
                waits = list(si.on_wait) if si is not None else []
                if len(waits) > 1:
                    changed = True
                    for j, w in enumerate(waits[:-1]):
                        nop = mybir.InstNoOp(
                            name=f"{inst.name}-sw{j}", ins=[], outs=[]
                        )
                        nop.engine = inst.engine
                        nop.sync_info = mybir.SyncInfo(
                            on_wait=[w], on_update=[]
                        )
                        new.append(nop)
                        n_split += 1
                    inst.sync_info = mybir.SyncInfo(
                        on_wait=[waits[-1]], on_update=list(si.on_update)
                    )
                new.append(inst)
            if changed:
                bb.instructions = new
    return n_split


def _build_program(debug=False):
    import concourse.bass as bass
    import concourse.mybir as mybir
    from concourse.tile import TileContext, add_dep_helper

    dt = mybir.dt
    AF = mybir.ActivationFunctionType

    nc = bass.Bass()

    xT_d = nc.dram_tensor("xT", [D, T], dt.bfloat16, kind="ExternalInput")
    wq_d = nc.dram_tensor("wq", [D, D], dt.bfloat16, kind="ExternalInput")
    wk_d = nc.dram_tensor("wk", [D, D], dt.bfloat16, kind="ExternalInput")
    wv_d = nc.dram_tensor("wv", [D, D], dt.bfloat16, kind="ExternalInput")
    wo_d = nc.dram_tensor("wo", [D, D], dt.bfloat16, kind="ExternalInput")
    bqs_d = nc.dram_tensor("bqs", [P, DC], dt.float32, kind="ExternalInput")
    bos_d = nc.dram_tensor("bos", [P, DC], dt.float32, kind="ExternalInput")
    bvb_d = nc.dram_tensor("bvb", [P, D], dt.float32, kind="ExternalInput")
    yT_d = nc.dram_tensor("yT", [D, T], dt.float32, kind="ExternalOutput")
    if debug:
        phiq_d = nc.dram_tensor("phiq_dump", [P, DC, T], dt.bfloat16, kind="ExternalOutput")
        kv_d = nc.dram_tensor("kv_dump", [P, 512], dt.float32, kind="ExternalOutput")
        m_d = nc.dram_tensor("m_dump", [P, DC, D], dt.bfloat16, kind="ExternalOutput")
        kproj_d = nc.dram_tensor("kproj_dump", [P, D], dt.float32, kind="ExternalOutput")
        vproj_d = nc.dram_tensor("vproj_dump", [P, D], dt.float32, kind="ExternalOutput")

    with TileContext(nc) as tc:
        with (
            tc.tile_pool(name="weights", bufs=1) as wpool,
            tc.tile_pool(name="phiq", bufs=1) as qpool,
            tc.tile_pool(name="msb", bufs=1) as mpool,
            tc.tile_pool(name="xin", bufs=3) as xpool,
            tc.tile_pool(name="kvtiles", bufs=6) as kvpool,
            tc.tile_pool(name="yout", bufs=2) as ypool,
        ):
            # ---- weight / const preload ----
            # wq + the first x pair come first (they gate the first matmuls);
            # x tiles stream on the gpsimd queue, weights on sync, wo (only
            # needed at the M stage) last.
            wq_sb = wpool.tile([P, DC, D], dt.bfloat16, tag="wq")
            wk_sb = wpool.tile([P, DC, D], dt.bfloat16, tag="wk")
            wv_sb = wpool.tile([P, DC, D], dt.bfloat16, tag="wv")
            wo_sb = wpool.tile([P, DC, D], dt.bfloat16, tag="wo")
            bq_sb = wpool.tile([P, DC], dt.float32, tag="bq")
            bo_sb = wpool.tile([P, DC], dt.float32, tag="bo")
            bv_sb = wpool.tile([P, D], dt.float32, tag="bv")
            nc.sync.dma_start(bq_sb[:], bqs_d[:])
            nc.sync.dma_start(bo_sb[:], bos_d[:])
            # wq in column halves: the first q matmuls (oc 0-3) only need the
            # first half, so PE starts ~3us earlier. bvb (0.5MB) is not needed
            # until the first kv sub-tile (~30us in), so it loads after wk.
            wq_r = wq_d.rearrange("(c p) o -> p c o", p=P)
            xT_r = xT_d.rearrange("(c p) t -> p c t", p=P)
            nc.sync.dma_start(wq_sb[:, :, 0:512], wq_r[:, :, 0:512])
            xt_pre = []
            for half in range(2):
                xt0 = xpool.tile([P, DC, TT], dt.bfloat16, tag="xt", name=f"xtpre{half}")
                nc.sync.dma_start(xt0[:], xT_r[:, :, half * TT : (half + 1) * TT])
                xt_pre.append(xt0)
            nc.sync.dma_start(wq_sb[:, :, 512:1024], wq_r[:, :, 512:1024])
            nc.sync.dma_start(wk_sb[:], wk_d.rearrange("(c p) o -> p c o", p=P))
            nc.sync.dma_start(bv_sb[:], bvb_d[:])
            nc.sync.dma_start(wv_sb[:], wv_d.rearrange("(c p) o -> p c o", p=P))
            nc.sync.dma_start(wo_sb[:], wo_d.rearrange("(c p) o -> p c o", p=P))

            phi_q = qpool.tile([P, DC, T], dt.bfloat16, tag="phiq")
            m_chunks = []
            for c in range(DC):
                m_chunk = mpool.tile(
                    [P, D], dt.bfloat16, tag=f"msb{c}", name=f"msb{c}"
                )
                m_chunks.append(m_chunk)
            kv_chunks = []
            for c in range(DC):
                kvc = mpool.tile(
                    [P, P], dt.bfloat16, tag=f"kvsb{c}", name=f"kvsb{c}"
                )
                kv_chunks.append(kvc)

            zz = wpool.tile([1, 640], dt.bfloat16, tag="zz")
            nc.vector.memset(zz[:], 0.0)
            # kv chunk off-diagonal blocks must be zero (block-diag repack)
            for c in range(DC):
                nc.vector.memset(kv_chunks[c][:], 0.0)

            with tc.tile_pool(name="ps_kv", bufs=1, space="PSUM") as pkv_pool:
                kv_ps = pkv_pool.tile([P, 512], dt.float32, tag="kvacc")
                # zero the whole kv bank once (sets has_written for every
                # element) so the 16 interleaved head slots can accumulate
                # with start=False; multiple start=True groups in one bank
                # clobber each other.
                nc.tensor.matmul(
                    kv_ps[:], lhsT=zz[:1, :P], rhs=zz[:1, P : P + 512],
                    start=True, stop=True, skip_group_check=True,
                )
                # warmup matmuls filling the startup DMA shadow: semantically
                # they re-write zeros over the (unused-yet) kv bank, but they
                # keep the PE p-state/HAM warm so the first real matmuls run
                # at full clock the moment weights land. N=128 keeps the
                # granularity fine so the last one barely delays real work.
                for w in range(88):
                    nc.tensor.matmul(
                        kv_ps[:, 0:128], lhsT=zz[:1, :P], rhs=zz[:1, P : P + 128],
                        start=True, stop=True, skip_group_check=True,
                    )
                nc.tensor.matmul(
                    kv_ps[:], lhsT=zz[:1, :P], rhs=zz[:1, P : P + 512],
                    start=True, stop=True, skip_group_check=True,
                )

                kv_pend = [None]

                def _emit_kv(pending, last):
                    phik_p, vsb_p = pending
                    for h in range(H):
                        r0 = (h % 2) * 64
                        c0 = (h // 2) * 64
                        nc.tensor.matmul(
                            kv_ps[r0 : r0 + 64, c0 : c0 + 64],
                            lhsT=vsb_p[:, h * 64 : (h + 1) * 64],
                            rhs=phik_p[:, h * 64 : (h + 1) * 64],
                            start=False,
                            stop=last and h == H - 1,
                            skip_group_check=True,
                        )

                with tc.tile_pool(name="ps_q", bufs=3, space="PSUM") as pq_pool:
                  with tc.tile_pool(name="ps_kvp", bufs=2, space="PSUM") as pkvp_pool:
                      # ---- q projection (feature-major out); both tiles of a
                      # pair share each stationary weight load. first_split runs
                      # tile A before tile B (pair 0: B's DMA still in flight).
                      # flush_after_oc0 emits the last kv matmuls between q
                      # chunks so the M stage overlaps the q stream.
                      def _q_section(pair, xts, first_split, post_oc=None):
                          def _drain(oc, half, psx):
                              tt = pair * 2 + half
                              pq_slice = phi_q[:, oc, tt * TT : (tt + 1) * TT]
                              nc.scalar.activation(
                                  pq_slice, psx[:], AF.Silu,
                                  bias=bq_sb[:, oc : oc + 1], scale=SCALE,
                              )
                              nc.vector.tensor_scalar_add(pq_slice, pq_slice, 1.0)

                          for oc in range(DC):
                              psA = pq_pool.tile([P, TT], dt.float32, tag="psq")
                              psB = pq_pool.tile([P, TT], dt.float32, tag="psq")
                              if first_split and oc == 0:
                                  last_a = None
                                  for d in range(DC):
                                      last_a = nc.tensor.matmul(
                                          psA[:],
                                          lhsT=wq_sb[:, d, oc * P : (oc + 1) * P],
                                          rhs=xts[0][:, d, :],
                                          start=(d == 0),
                                          stop=(d == DC - 1),
                                      )
                                  # bridge the B-tile DMA wait with warmup
                                  # zero-rewrites of the (still unused) kv bank;
                                  # dep-pinned after the A matmuls so the
                                  # scheduler cannot hoist them earlier
                                  for w in range(14):
                                      dmy = nc.tensor.matmul(
                                          kv_ps[:, 0:128],
                                          lhsT=zz[:1, :P],
                                          rhs=zz[:1, P : P + 128],
                                          start=True, stop=True,
                                          skip_group_check=True,
                                      )
                                      add_dep_helper(
                                          dmy.ins, last_a.ins, sync=False,
                                          reason="bridge dummies after A matmuls",
                                      )
                                  dmy = nc.tensor.matmul(
                                      kv_ps[:], lhsT=zz[:1, :P],
                                      rhs=zz[:1, P : P + 512],
                                      start=True, stop=True,
                                      skip_group_check=True,
                                  )
                                  add_dep_helper(
                                      dmy.ins, last_a.ins, sync=False,
                                      reason="bridge dummies after A matmuls",
                                  )
                                  for d in range(DC):
                                      nc.tensor.matmul(
                                          psB[:],
                                          lhsT=wq_sb[:, d, oc * P : (oc + 1) * P],
                                          rhs=xts[1][:, d, :],
                                          start=(d == 0),
                                          stop=(d == DC - 1),
                                      )
                              else:
                                  for d in range(DC):
                                      nc.tensor.matmul(
                                          psA[:],
                                          lhsT=wq_sb[:, d, oc * P : (oc + 1) * P],
                                          rhs=xts[0][:, d, :],
                                          start=(d == 0),
                                          stop=(d == DC - 1),
                                      )
                                      nc.tensor.matmul(
                                          psB[:],
                                          lhsT=wq_sb[:, d, oc * P : (oc + 1) * P],
                                          rhs=xts[1][:, d, :],
                                          start=(d == 0),
                                          stop=(d == DC - 1),
                                      )
                              _drain(oc, 0, psA)
                              _drain(oc, 1, psB)
                              if post_oc is not None and oc in post_oc:
                                  post_oc[oc]()

                      # ---- k,v projections (token-major) + kv accumulation.
                      # The 16 kv-accumulate matmuls for a sub-tile are emitted
                      # one sub-tile LATE so PE never waits on silu/+bv. ----
                      def _kvproj_section(pair, xts):
                          for half in range(2):
                              tt = pair * 2 + half
                              xt = xts[half]
                              for sub in range(NSUB):
                                  pk = pkvp_pool.tile([P, D], dt.float32, tag="pkv")
                                  pv = pkvp_pool.tile([P, D], dt.float32, tag="pkv")
                                  xs = xt[:, :, sub * P : (sub + 1) * P]
                                  for d in range(DC):
                                      for n in range(2):
                                          nc.tensor.matmul(
                                              pk[:, n * 512 : (n + 1) * 512],
                                              lhsT=xs[:, d, :],
                                              rhs=wk_sb[:, d, n * 512 : (n + 1) * 512],
                                              start=(d == 0),
                                              stop=(d == DC - 1),
                                          )
                                      for n in range(2):
                                          nc.tensor.matmul(
                                              pv[:, n * 512 : (n + 1) * 512],
                                              lhsT=xs[:, d, :],
                                              rhs=wv_sb[:, d, n * 512 : (n + 1) * 512],
                                              start=(d == 0),
                                              stop=(d == DC - 1),
                                          )
                                  if debug and tt == 0 and sub == 0:
                                      kpf = mpool.tile([P, D], dt.float32, tag="kpdump")
                                      vpf = mpool.tile([P, D], dt.float32, tag="vpdump")
                                      nc.vector.tensor_copy(out=kpf[:], in_=pk[:])
                                      nc.vector.tensor_copy(out=vpf[:], in_=pv[:])
                                      nc.sync.dma_start(kproj_d[:], kpf[:])
                                      nc.sync.dma_start(vproj_d[:], vpf[:])
                                  phik = kvpool.tile([P, D], dt.bfloat16, tag="phik")
                                  vsb = kvpool.tile([P, D], dt.bfloat16, tag="vsb")
                                  nc.scalar.activation(
                                      phik[:], pk[:], AF.Silu, scale=SCALE
                                  )
                                  nc.vector.tensor_scalar_add(phik[:], phik[:], 1.0)
                                  nc.vector.tensor_add(vsb[:], pv[:], bv_sb[:])
                                  if kv_pend[0] is not None:
                                      _emit_kv(kv_pend[0], False)
                                  kv_pend[0] = (phik, vsb)

                      for pair in range(NTT // 2):
                          if pair == 0:
                              xts = xt_pre
                          else:
                              xts = []
                              for half in range(2):
                                  tt = pair * 2 + half
                                  xt = xpool.tile([P, DC, TT], dt.bfloat16, tag="xt")
                                  nc.gpsimd.dma_start(
                                      xt[:], xT_r[:, :, tt * TT : (tt + 1) * TT]
                                  )
                                  xts.append(xt)

                          if pair == NTT // 2 - 1:
                              # last pair: kvproj first, then q. The kv flush,
                              # repack copies, M matmuls (psum borrowed from the
                              # drained kvproj pool — no extra banks) and
                              # m-chunk copies are spread across the q chunk
                              # boundaries, fully hidden under the 27us of q
                              # matmuls with no engine head-of-line blocking.
                              _kvproj_section(pair, xts)

                              def _hook_flush():
                                  _emit_kv(kv_pend[0], True)
                                  kv_pend[0] = None
                                  for c in range(DC):
                                      if c % 2 == 0:
                                          nc.vector.tensor_copy(
                                              out=kv_chunks[c][0:64, 0:64],
                                              in_=kv_ps[0:64, c * 64 : (c + 1) * 64],
                                          )
                                          nc.vector.tensor_copy(
                                              out=kv_chunks[c][64:128, 64:128],
                                              in_=kv_ps[64:128, c * 64 : (c + 1) * 64],
                                          )
                                      else:
                                          nc.scalar.copy(
                                              out=kv_chunks[c][0:64, 0:64],
                                              in_=kv_ps[0:64, c * 64 : (c + 1) * 64],
                                          )
                                          nc.scalar.copy(
                                              out=kv_chunks[c][64:128, 64:128],
                                              in_=kv_ps[64:128, c * 64 : (c + 1) * 64],
                                          )

                              def _mk_hook_m(c0):
                                  def _hook():
                                      for c in (c0, c0 + 1):
                                          pm = pkvp_pool.tile(
                                              [P, D], dt.float32, tag="pkv"
                                          )
                                          for n in range(2):
                                              nc.tensor.matmul(
                                                  pm[:, n * 512 : (n + 1) * 512],
                                                  lhsT=kv_chunks[c][:],
                                                  rhs=wo_sb[:, c, n * 512 : (n + 1) * 512],
                                                  start=True,
                                                  stop=True,
                                              )
                                          nc.vector.tensor_copy(
                                              out=m_chunks[c][:, 0:512],
                                              in_=pm[:, 0:512],
                                          )
                                          nc.scalar.copy(
                                              out=m_chunks[c][:, 512:1024],
                                              in_=pm[:, 512:1024],
                                          )
                                  return _hook

                              hooks = {0: _hook_flush}
                              for c0 in range(0, DC, 2):
                                  hooks[1 + c0 // 2] = _mk_hook_m(c0)
                              _q_section(pair, xts, False, hooks)
                          else:
                              _q_section(pair, xts, pair == 0)
                              _kvproj_section(pair, xts)

                      if kv_pend[0] is not None:
                          _emit_kv(kv_pend[0], True)
                          kv_pend[0] = None

                  if debug:
                      kvf = mpool.tile([P, 512], dt.float32, tag="kvdump")
                      nc.vector.tensor_copy(out=kvf[:], in_=kv_ps[:])
                      nc.sync.dma_start(kv_d[:], kvf[:])
                  # ---- phase 2: yT = M.T @ phi_q + bo ----
                  # [128,1024] psum tiles (bufs=4): the whole-tile RAW window is 16
                  # matmuls, so each tile's drain overlaps the next tiles' matmuls
                  # and the kernel tail is just one small tile's drain.
                  with tc.tile_pool(name="ps_y", bufs=2, space="PSUM") as py_pool:
                      for oc in range(DC):
                          for qb in range(4):
                              if oc == DC - 1 and qb == 3:
                                  # very last block: two independent [128,512] psum
                                  # tiles so the final drain is one small piece that
                                  # starts 8 matmuls before the end
                                  for i in range(2):
                                      pyf = py_pool.tile([P, 512], dt.float32, tag="py")
                                      for j in range(DC):
                                          f = (oc * 4 + qb + j) % DC
                                          nc.tensor.matmul(
                                              pyf[:],
                                              lhsT=m_chunks[f][:, oc * P : (oc + 1) * P],
                                              rhs=phi_q[
                                                  :, f, qb * 1024 + i * 512 : qb * 1024 + (i + 1) * 512
                                              ],
                                              start=(j == 0),
                                              stop=(j == DC - 1),
                                          )
                                      ysf = ypool.tile(
                                          [P, 512], dt.float32, tag=f"ys{i}"
                                      )
                                      if i == 0:
                                          nc.scalar.activation(
                                              ysf[:], pyf[:], AF.Identity,
                                              bias=bo_sb[:, oc : oc + 1], scale=1.0,
                                          )
                                          nc.sync.dma_start(
                                              yT_d[
                                                  oc * P : (oc + 1) * P,
                                                  qb * 1024 : qb * 1024 + 512,
                                              ],
                                              ysf[:],
                                          )
                                      else:
                                          nc.vector.tensor_scalar_add(
                                              ysf[:], pyf[:], bo_sb[:, oc : oc + 1]
                                          )
                                          nc.gpsimd.dma_start(
                                              yT_d[
                                                  oc * P : (oc + 1) * P,
                                                  qb * 1024 + 512 : (qb + 1) * 1024,
                                              ],
                                              ysf[:],
                                          )
                                  continue
                              py = py_pool.tile([P, 1024], dt.float32, tag="py")
                              # rotated f-order: successive tiles start on different
                              # M chunks, so phase 2 begins as soon as the first
                              # chunk copy lands and the rest overlap these matmuls
                              for j in range(DC):
                                  f = (oc * 4 + qb + j) % DC
                                  for i in range(2):
                                      nc.tensor.matmul(
                                          py[:, i * 512 : (i + 1) * 512],
                                          lhsT=m_chunks[f][:, oc * P : (oc + 1) * P],
                                          rhs=phi_q[
                                              :, f, qb * 1024 + i * 512 : qb * 1024 + (i + 1) * 512
                                          ],
                                          start=(j == 0),
                                          stop=(j == DC - 1),
                                      )
                              # drain in two 512 pieces on ACT+sync / DVE+gpsimd
                              ys0 = ypool.tile([P, 512], dt.float32, tag="ys0")
                              nc.scalar.activation(
                                  ys0[:], py[:, 0:512],
                                  AF.Identity, bias=bo_sb[:, oc : oc + 1], scale=1.0,
                              )
                              nc.sync.dma_start(
                                  yT_d[
                                      oc * P : (oc + 1) * P,
                                      qb * 1024 : qb * 1024 + 512,
                                  ],
                                  ys0[:],
                              )
                              ys1 = ypool.tile([P, 512], dt.float32, tag="ys1")
                              nc.vector.tensor_scalar_add(
                                  ys1[:], py[:, 512:1024], bo_sb[:, oc : oc + 1]
                              )
                              nc.gpsimd.dma_start(
                                  yT_d[
                                      oc * P : (oc + 1) * P,
                                      qb * 1024 + 512 : (qb + 1) * 1024,
                                  ],
                                  ys1[:],
                              )

            if debug:
                nc.sync.dma_start(phiq_d[:], phi_q[:])
                for c in range(DC):
                    nc.sync.dma_start(m_d[:, c, :], m_chunks[c][:])
    _split_multi_waits(nc)
    return nc


def _get_program(debug=False):
    key = ("nc", debug)
    if key not in _CACHE:
        _CACHE[key] = _build_program(debug)
    return _CACHE[key]


def _prep_shared(Wq, bq, Wk, Wv, bv, Wo, bo):
    shared = {
        "wq": np.ascontiguousarray(Wq.T).astype(_BF16),
        "wk": np.ascontiguousarray(Wk.T).astype(_BF16),
        "wv": np.ascontiguousarray(Wv.T).astype(_BF16),
        "wo": np.ascontiguousarray(Wo.T).astype(_BF16),
        "bqs": np.ascontiguousarray(
            (SCALE * bq).astype(np.float32).reshape(DC, P).T
        ),
        "bos": np.ascontiguousarray(bo.astype(np.float32).reshape(DC, P).T),
        "bvb": np.ascontiguousarray(
            np.broadcast_to(bv.astype(np.float32), (P, D))
        ),
    }
    return shared


def _run(in_maps, trace=False, debug=False, **kw):
    from concourse.bass_utils import run_bass_kernel_spmd

    nc = _get_program(debug)
    return run_bass_kernel_spmd(nc, in_maps, list(range(NCORES)), trace=trace, **kw)


def kernel(x, Wq, bq, Wk, Wv, bv, Wo, bo):
    x = np.asarray(x, dtype=np.float32)
    assert x.shape == (B, T, D), x.shape
    shared = _prep_shared(
        np.asarray(Wq, np.float32), np.asarray(bq, np.float32),
        np.asarray(Wk, np.float32), np.asarray(Wv, np.float32),
        np.asarray(bv, np.float32), np.asarray(Wo, np.float32),
        np.asarray(bo, np.float32),
    )
    in_maps = []
    for b in range(B):
        m = dict(shared)
        m["xT"] = np.ascontiguousarray(x[b].T).astype(_BF16)
        in_maps.append(m)

    res = _run(in_maps)
    out = np.empty((B, T, D), np.float32)
    for b in range(B):
        out[b] = res.results[b]["yT"].T
    return out



# revision 10
# speedup vs baseline: 2.4303x; 2.4303x over previous
"""Linear attention (silu+1 feature map) MultiHeadAttention kernel for 8x TRN2.

Sharding: data-parallel over batch (B=8 -> 1 batch element per NeuronCore).

fp8 DoubleRow formulation (all big GEMMs at fp8 2x rate, fp32 PSUM):

  stage 1 (stream 512-token tiles, feature-major x = xT8):
    fT[o,t] = silu(s*(WqT.T @ xT) + s*bq)        f = phi_q - 1, fp8 [P,DC,T]
    g[t,d]  = silu(s*(xT.T @ WkT))               g = phi_k - 1, fp8 [P,32,D]
    csg[d] += ones.T-row reductions of g          (DR matmuls, column form)
  A-GEMM (token-major x = xt8, 2 d-half passes, 8 PSUM banks):
    At[E,d] = sum_t x[t,E]*g[t,d]                 DR fp8
    Asb     = At/32 + csx[E]                      (csx = exact host colsum of x)
  kv assembly (bf16):
    kv_h[e,d] = Wv_h @ Asb[:,d_h] + bv_h (x) csg_h   (+ T*bv_h bias at repack)
    (identity: kv = phi_k^T v = Wv@(colsum_x + g^T x) + bv*(T + colsum_g))
  M stage:
    M_h[d,o] = kv_h.T @ (Wo_h/2)  -> m8 = fp8(2*pm) = fp8(M)
    colsum_M via rowsum(kv) hi/lo bf16 split @ Wo   -> phase-2 bias
  phase 2:
    yT[o,t] = m8.T @ f8 + (colsum_M + bo)         DR fp8; out bf16
    (identity: phi_q @ kv @ Wo = f @ M + colsum(M))

Host: fp8 casts (x*32 both layouts, W.T*1024), Wv.T/Wo.T*0.5 bf16, exact
colsum_x, T*bv, bias prep. Output bf16 -> fp32 on host.
"""

import numpy as np
import ml_dtypes

B, T, D = 8, 4096, 1024
H, DH = 16, 64
SCALE = float(DH ** -0.25)
NCORES = 8
P = 128
DC = D // P          # 8 feature chunks
TT = 512             # token tile (stage 1)
NTT = T // TT        # 8 token tiles
NSUB = TT // P       # 4 sub-tiles of 128 tokens
NG = T // P          # 32 token-major g/x subtiles
XS = 32.0            # x fp8 prescale
WS = 1024.0          # Wq/Wk fp8 prescale
QSCALE = SCALE / (XS * WS)
ASCALE = 1.0 / XS    # Asb descale

_BF16 = ml_dtypes.bfloat16
_F8 = ml_dtypes.float8_e4m3

_CACHE = {}


def _split_multi_waits(nc):
    """walrus in this container only encodes ONE sync-wait command per
    instruction. Hoist extra waits onto injected same-engine NOPs placed
    immediately before the instruction (program order on the engine queue
    makes this semantically identical)."""
    import concourse.mybir as mybir

    n_split = 0
    for fn in nc.m.functions:
        for bb in fn.blocks:
            new = []
            changed = False
            for inst in bb.instructions:
                si = inst.sync_info
                waits = list(si.on_wait) if si is not None else []
                if len(waits) > 1:
                    changed = True
                    for j, w in enumerate(waits[:-1]):
                        nop = mybir.InstNoOp(
                            name=f"{inst.name}-sw{j}", ins=[], outs=[]
                        )
                        nop.engine = inst.engine
                        nop.sync_info = mybir.SyncInfo(
                            on_wait=[w], on_update=[]
                        )
                        new.append(nop)
                        n_split += 1
                    inst.sync_info = mybir.SyncInfo(
                        on_wait=[waits[-1]], on_update=list(si.on_update)
                    )
                new.append(inst)
            if changed:
                bb.instructions = new
    return n_split


def _build_program(debug=False):
    import concourse.bass as bass
    import concourse.mybir as mybir
    from concourse.tile import TileContext
    from concourse.masks import make_identity

    dt = mybir.dt
    AF = mybir.ActivationFunctionType
    DR = mybir.MatmulPerfMode.DoubleRow
    ALU = mybir.AluOpType

    nc = bass.Bass()

    xT8_d = nc.dram_tensor("xT8", [D, T], dt.float8e4, kind="ExternalInput")
    xt8_d = nc.dram_tensor("xt8", [T, D], dt.float8e4, kind="ExternalInput")
    wq8_d = nc.dram_tensor("wq8", [D, D], dt.float8e4, kind="ExternalInput")
    wk8_d = nc.dram_tensor("wk8", [D, D], dt.float8e4, kind="ExternalInput")
    wvb_d = nc.dram_tensor("wvb", [D, D], dt.bfloat16, kind="ExternalInput")
    wob_d = nc.dram_tensor("wob", [D, D], dt.bfloat16, kind="ExternalInput")
    bqs_d = nc.dram_tensor("bqs", [P, DC], dt.float32, kind="ExternalInput")
    bos_d = nc.dram_tensor("bos", [P, DC], dt.float32, kind="ExternalInput")
    csx_d = nc.dram_tensor("csx", [P, DC], dt.float32, kind="ExternalInput")
    tbv_d = nc.dram_tensor("tbv", [P, DC], dt.float32, kind="ExternalInput")
    bvr_d = nc.dram_tensor("bvr", [1, D], dt.bfloat16, kind="ExternalInput")
    csg_scr_d = nc.dram_tensor("csg_scr", [DC, P], dt.bfloat16, kind="Internal")
    yT_d = nc.dram_tensor("yT", [D, T], dt.bfloat16, kind="ExternalOutput")
    if debug:
        f_dump = nc.dram_tensor("f_dump", [P, DC, T], dt.float8e4, kind="ExternalOutput")
        g_dump = nc.dram_tensor("g_dump", [P, NG, D], dt.float8e4, kind="ExternalOutput")
        a_dump = nc.dram_tensor("a_dump", [P, DC, D], dt.bfloat16, kind="ExternalOutput")
        kv_dump = nc.dram_tensor("kv_dump", [P, DC, P], dt.bfloat16, kind="ExternalOutput")
        m_dump = nc.dram_tensor("m_dump", [P, DC, D], dt.float8e4, kind="ExternalOutput")
        csg_dump = nc.dram_tensor("csg_dump", [1, D], dt.bfloat16, kind="ExternalOutput")
        bias_dump = nc.dram_tensor("bias_dump", [P, DC], dt.float32, kind="ExternalOutput")

    with TileContext(nc) as tc:
        with (
            tc.tile_pool(name="weights", bufs=1) as wpool,
            tc.tile_pool(name="fstore", bufs=1) as fpool,
            tc.tile_pool(name="gstore", bufs=1) as gpool,
            tc.tile_pool(name="xtok", bufs=1) as xkpool,
            tc.tile_pool(name="xin", bufs=3) as xpool,
            tc.tile_pool(name="asb", bufs=1) as apool,
            tc.tile_pool(name="msb", bufs=1) as mpool,
            tc.tile_pool(name="yout", bufs=4) as ypool,
        ):
            # ---- weight / const preload ----
            wq_sb = wpool.tile([P, DC, D], dt.float8e4, tag="wq")
            wk_sb = wpool.tile([P, DC, D], dt.float8e4, tag="wk")
            wv_sb = wpool.tile([P, DC, D], dt.bfloat16, tag="wv")
            wo_sb = wpool.tile([P, DC, D], dt.bfloat16, tag="wo")
            bq_sb = wpool.tile([P, DC], dt.float32, tag="bq")
            bo_sb = wpool.tile([P, DC], dt.float32, tag="bo")
            csx_sb = wpool.tile([P, DC], dt.float32, tag="csx")
            tbv_sb = wpool.tile([P, DC], dt.float32, tag="tbv")
            bvr_sb = wpool.tile([1, D], dt.bfloat16, tag="bvr")
            ident = wpool.tile([P, P], dt.bfloat16, tag="ident")

            nc.sync.dma_start(bq_sb[:], bqs_d[:])
            nc.sync.dma_start(bo_sb[:], bos_d[:])
            wq_r = wq8_d.rearrange("(c p) o -> p c o", p=P)
            xT_r = xT8_d.rearrange("(c p) t -> p c t", p=P)
            xt_r = xt8_d.rearrange("(s p) d -> p s d", p=P)
            nc.sync.dma_start(wq_sb[:, :, 0:512], wq_r[:, :, 0:512])
            xt_pre = []
            for half in range(2):
                xt0 = xpool.tile([P, DC, TT], dt.float8e4, tag="xt", name=f"xtpre{half}")
                nc.sync.dma_start(xt0[:], xT_r[:, :, half * TT : (half + 1) * TT])
                xt_pre.append(xt0)
            nc.sync.dma_start(wq_sb[:, :, 512:1024], wq_r[:, :, 512:1024])
            nc.sync.dma_start(wk_sb[:], wk8_d.rearrange("(c p) o -> p c o", p=P))

            # token-major x for the A-GEMM: stream on the gpsimd queue early
            xtok_sb = xkpool.tile([P, NG, D], dt.float8e4, tag="xtok")
            for qtr in range(4):
                nc.gpsimd.dma_start(
                    xtok_sb[:, qtr * 8 : (qtr + 1) * 8, :],
                    xt_r[:, qtr * 8 : (qtr + 1) * 8, :],
                )
            nc.sync.dma_start(wv_sb[:], wvb_d.rearrange("(c p) o -> p c o", p=P))
            nc.sync.dma_start(wo_sb[:], wob_d.rearrange("(c p) o -> p c o", p=P))
            nc.sync.dma_start(csx_sb[:], csx_d[:])
            nc.sync.dma_start(tbv_sb[:], tbv_d[:])
            nc.sync.dma_start(bvr_sb[:], bvr_d[:])
            make_identity(nc, ident[:])

            f8_sb = fpool.tile([P, DC, T], dt.float8e4, tag="f8")
            g_sb = gpool.tile([P, NG, D], dt.float8e4, tag="g8")
            asb = apool.tile([P, DC, D], dt.bfloat16, tag="asb")
            m8_sb = mpool.tile([P, DC, D], dt.float8e4, tag="m8")
            kv_chunks = []
            for c in range(DC):
                kvc = mpool.tile([P, P], dt.bfloat16, tag=f"kvsb{c}", name=f"kvsb{c}")
                kv_chunks.append(kvc)
            csg_fm = mpool.tile([P, DC], dt.bfloat16, tag="csgfm")
            csg_row = mpool.tile([1, D], dt.bfloat16, tag="csgrow")
            rs_f32 = mpool.tile([P, DC], dt.float32, tag="rsf32")
            rs_hi = mpool.tile([P, DC], dt.bfloat16, tag="rshi")
            rs_lo = mpool.tile([P, DC], dt.bfloat16, tag="rslo")
            bias_fin = mpool.tile([P, DC], dt.float32, tag="biasfin")

            zz = wpool.tile([1, 640], dt.bfloat16, tag="zz")
            nc.vector.memset(zz[:], 0.0)
            for c in range(DC):
                nc.vector.memset(kv_chunks[c][:], 0.0)

            # ================= stage 1: projections =================
            with (
                tc.tile_pool(name="ps_q", bufs=3, space="PSUM") as pq_pool,
                tc.tile_pool(name="ps_k", bufs=2, space="PSUM") as pk_pool,
                tc.tile_pool(name="ps_misc", bufs=1, space="PSUM") as pmisc_pool,
            ):
                warm = pmisc_pool.tile([P, 512], dt.float32, tag="warm")
                # keep PE p-state warm under the startup DMA shadow; the
                # final full-tile zero write also clears has_written for the
                # csg accumulation slots (cols 0:8).
                nc.tensor.matmul(
                    warm[:], lhsT=zz[:1, :P], rhs=zz[:1, P : P + 512],
                    start=True, stop=True, skip_group_check=True,
                )
                for w in range(88):
                    nc.tensor.matmul(
                        warm[:, 0:128], lhsT=zz[:1, :P], rhs=zz[:1, P : P + 128],
                        start=True, stop=True, skip_group_check=True,
                    )
                nc.tensor.matmul(
                    warm[:], lhsT=zz[:1, :P], rhs=zz[:1, P : P + 512],
                    start=True, stop=True, skip_group_check=True,
                )

                def _q_section(pair, xts):
                    # f[o-feat, token] = silu(SCALE*q + SCALE*bq), fp8 store
                    for oc in range(DC):
                        psA = pq_pool.tile([P, TT], dt.float32, tag="psq")
                        psB = pq_pool.tile([P, TT], dt.float32, tag="psq")
                        for j in range(4):
                            nc.tensor.matmul(
                                psA[:],
                                lhsT=wq_sb[:, 2 * j : 2 * j + 2, oc * P : (oc + 1) * P],
                                rhs=xts[0][:, 2 * j : 2 * j + 2, :],
                                start=(j == 0), stop=(j == 3), perf_mode=DR,
                            )
                            nc.tensor.matmul(
                                psB[:],
                                lhsT=wq_sb[:, 2 * j : 2 * j + 2, oc * P : (oc + 1) * P],
                                rhs=xts[1][:, 2 * j : 2 * j + 2, :],
                                start=(j == 0), stop=(j == 3), perf_mode=DR,
                            )
                        for half, psx in ((0, psA), (1, psB)):
                            tt = pair * 2 + half
                            nc.scalar.activation(
                                f8_sb[:, oc, tt * TT : (tt + 1) * TT], psx[:],
                                AF.Silu, bias=bq_sb[:, oc : oc + 1], scale=QSCALE,
                            )

                def _k_section(pair, xts):
                    # g[token, d-feat] = silu(SCALE*k), fp8 store token-major
                    for half in range(2):
                        tt = pair * 2 + half
                        xt = xts[half]
                        for sub in range(NSUB):
                            gs = tt * NSUB + sub     # global 128-token subtile
                            pk = pk_pool.tile([P, D], dt.float32, tag="pk")
                            for j in range(4):
                                for n in range(2):
                                    nc.tensor.matmul(
                                        pk[:, n * 512 : (n + 1) * 512],
                                        lhsT=xt[:, 2 * j : 2 * j + 2, sub * P : (sub + 1) * P],
                                        rhs=wk_sb[:, 2 * j : 2 * j + 2, n * 512 : (n + 1) * 512],
                                        start=(j == 0), stop=(j == 3), perf_mode=DR,
                                    )
                            nc.scalar.activation(
                                g_sb[:, gs, :], pk[:], AF.Silu, scale=QSCALE,
                            )
                            if gs % 2 == 1:
                                # csg[d] += sum_t g over this subtile pair
                                # (column form [128, 8] in the warm bank)
                                sp = gs // 2
                                for dcc in range(DC):
                                    nc.tensor.matmul(
                                        warm[:, dcc : dcc + 1],
                                        lhsT=g_sb[:, 2 * sp : 2 * sp + 2, dcc * P : (dcc + 1) * P],
                                        rhs=ones8[:, 0:2, 0:1],
                                        start=False, stop=(gs == NG - 1 and dcc == DC - 1),
                                        perf_mode=DR, skip_group_check=True,
                                    )

                # dim-1 step must be a multiple of 16B for DoubleRow APs
                ones8 = wpool.tile([P, 2, 16], dt.float8e4, tag="ones8")
                nc.vector.memset(ones8[:], 1.0)

                for pair in range(NTT // 2):
                    if pair == 0:
                        xts = xt_pre
                    else:
                        xts = []
                        for half in range(2):
                            tt = pair * 2 + half
                            xt = xpool.tile([P, DC, TT], dt.float8e4, tag="xt")
                            nc.gpsimd.dma_start(
                                xt[:], xT_r[:, :, tt * TT : (tt + 1) * TT]
                            )
                            xts.append(xt)
                    if pair == NTT // 2 - 1:
                        _k_section(pair, xts)
                        _q_section(pair, xts)
                    else:
                        _q_section(pair, xts)
                        _k_section(pair, xts)

                # csg column -> bf16 sbuf (for PE transpose)
                nc.vector.tensor_copy(out=csg_fm[:], in_=warm[:, 0:DC])

            # ================= A-GEMM: At[E,d] = x^T g =================
            with tc.tile_pool(name="ps_a", bufs=1, space="PSUM") as pa_pool:
                pa = [
                    pa_pool.tile([P, 512], dt.float32, tag=f"pa{e}", name=f"pa{e}")
                    for e in range(DC)
                ]
                for dh in range(2):
                    for s in range(NG // 2):
                        for e in range(DC):
                            nc.tensor.matmul(
                                pa[e][:],
                                lhsT=xtok_sb[:, 2 * s : 2 * s + 2, e * P : (e + 1) * P],
                                rhs=g_sb[:, 2 * s : 2 * s + 2, dh * 512 : (dh + 1) * 512],
                                start=(s == 0), stop=(s == NG // 2 - 1),
                                perf_mode=DR,
                            )
                    for e in range(DC):
                        # Asb = At/32 + csx[E]  (exact colsum_x folded in)
                        nc.vector.tensor_scalar(
                            out=asb[:, e, dh * 512 : (dh + 1) * 512],
                            in0=pa[e][:],
                            scalar1=ASCALE,
                            scalar2=csx_sb[:, e : e + 1],
                            op0=mybir.AluOpType.mult,
                            op1=mybir.AluOpType.add,
                        )

            # ============ kv assembly + M stage + phase-2 bias ============
            with (
                tc.tile_pool(name="ps_kv", bufs=1, space="PSUM") as pkv_pool,
                tc.tile_pool(name="ps_m", bufs=2, space="PSUM") as pm_pool,
                tc.tile_pool(name="ps_bias", bufs=1, space="PSUM") as pb_pool,
                tc.tile_pool(name="ps_t", bufs=1, space="PSUM") as pt_pool,
            ):
                kv_ps = pkv_pool.tile([P, 512], dt.float32, tag="kvacc")
                nc.tensor.matmul(
                    kv_ps[:], lhsT=zz[:1, :P], rhs=zz[:1, P : P + 512],
                    start=True, stop=True, skip_group_check=True,
                )
                # csg -> row form via PE transpose + small sbuf DMA
                csgT_ps = pt_pool.tile([DC, P], dt.bfloat16, tag="csgT")
                nc.tensor.transpose(csgT_ps[:], csg_fm[:], ident[:])
                csgT_sb = mpool.tile([DC, P], dt.bfloat16, tag="csgTsb")
                nc.vector.tensor_copy(out=csgT_sb[:], in_=csgT_ps[:])
                nc.sync.dma_start(csg_scr_d[:], csgT_sb[:])
                nc.sync.dma_start(
                    csg_row[:], csg_scr_d.rearrange("c p -> (c p)")
                )

                # kv_h[e,d] = sum_E Wv[e,E] * Asb[E,d]
                for e in range(DC):
                    for h in range(H):
                        r0 = (h % 2) * 64
                        c0 = (h // 2) * 64
                        nc.tensor.matmul(
                            kv_ps[r0 : r0 + 64, c0 : c0 + 64],
                            lhsT=wv_sb[:, e, h * 64 : (h + 1) * 64],
                            rhs=asb[:, e, h * 64 : (h + 1) * 64],
                            start=False, stop=False, skip_group_check=True,
                        )
                # + bv (x) csg  (rank-1 per head)
                for h in range(H):
                    r0 = (h % 2) * 64
                    c0 = (h // 2) * 64
                    nc.tensor.matmul(
                        kv_ps[r0 : r0 + 64, c0 : c0 + 64],
                        lhsT=bvr_sb[0:1, h * 64 : (h + 1) * 64],
                        rhs=csg_row[0:1, h * 64 : (h + 1) * 64],
                        start=False, stop=(h == H - 1), skip_group_check=True,
                    )
                # repack to block-diag chunks, adding the T*bv[e] bias
                for c in range(DC):
                    if c % 2 == 0:
                        nc.vector.tensor_scalar_add(
                            kv_chunks[c][0:64, 0:64],
                            kv_ps[0:64, c * 64 : (c + 1) * 64],
                            tbv_sb[0:64, c : c + 1],
                        )
                        nc.vector.tensor_scalar_add(
                            kv_chunks[c][64:128, 64:128],
                            kv_ps[64:128, c * 64 : (c + 1) * 64],
                            tbv_sb[64:128, c : c + 1],
                        )
                    else:
                        nc.scalar.activation(
                            kv_chunks[c][0:64, 0:64],
                            kv_ps[0:64, c * 64 : (c + 1) * 64],
                            AF.Identity, bias=tbv_sb[0:64, c : c + 1], scale=1.0,
                        )
                        nc.scalar.activation(
                            kv_chunks[c][64:128, 64:128],
                            kv_ps[64:128, c * 64 : (c + 1) * 64],
                            AF.Identity, bias=tbv_sb[64:128, c : c + 1], scale=1.0,
                        )

                # rowsum(kv) hi/lo split -> colsum(M) bias for phase 2
                for c in range(DC):
                    nc.vector.tensor_reduce(
                        rs_f32[:, c : c + 1], kv_chunks[c][:, :],
                        mybir.AxisListType.X, mybir.AluOpType.add,
                    )
                nc.vector.tensor_copy(out=rs_hi[:], in_=rs_f32[:])
                nc.vector.tensor_tensor(
                    rs_lo[:], rs_f32[:], rs_hi[:], mybir.AluOpType.subtract,
                )
                bias_ps = pb_pool.tile([P, DC], dt.float32, tag="biasps")
                for oc in range(DC):
                    for ci in range(2 * DC):
                        c, rs = ci // 2, (rs_hi if ci % 2 == 0 else rs_lo)
                        nc.tensor.matmul(
                            bias_ps[:, oc : oc + 1],
                            lhsT=wo_sb[:, c, oc * P : (oc + 1) * P],
                            rhs=rs[:, c : c + 1],
                            start=(ci == 0), stop=(ci == 2 * DC - 1),
                        )
                # bias_fin = 2*bias_ps + bo   (wo was halved on host)
                nc.vector.tensor_scalar_mul(bias_fin[:], bias_ps[:], 2.0)
                nc.vector.tensor_tensor(
                    bias_fin[:], bias_fin[:], bo_sb[:], mybir.AluOpType.add,
                )

                # M stage: m8 = fp8(2 * kv_chunks.T @ (Wo/2))
                for c in range(DC):
                    pm = pm_pool.tile([P, D], dt.float32, tag="pm")
                    for n in range(2):
                        nc.tensor.matmul(
                            pm[:, n * 512 : (n + 1) * 512],
                            lhsT=kv_chunks[c][:],
                            rhs=wo_sb[:, c, n * 512 : (n + 1) * 512],
                            start=True, stop=True,
                        )
                    nc.vector.tensor_scalar_mul(m8_sb[:, c, :], pm[:], 2.0)

            if debug:
                nc.sync.dma_start(f_dump[:], f8_sb[:])
                nc.sync.dma_start(g_dump[:], g_sb[:])
                nc.sync.dma_start(a_dump[:], asb[:])
                for c in range(DC):
                    nc.sync.dma_start(kv_dump[:, c, :], kv_chunks[c][:])
                nc.sync.dma_start(m_dump[:], m8_sb[:])
                nc.sync.dma_start(csg_dump[:], csg_row[:])
                nc.sync.dma_start(bias_dump[:], bias_fin[:])

            # ================= phase 2: yT = m8.T @ f8 + bias =================
            with tc.tile_pool(name="ps_y", bufs=3, space="PSUM") as py_pool:
                for oc in range(DC):
                    for qb in range(4):
                        py = py_pool.tile([P, 1024], dt.float32, tag="py")
                        for jj in range(4):
                            pr = (oc + qb + jj) % 4
                            for i in range(2):
                                nc.tensor.matmul(
                                    py[:, i * 512 : (i + 1) * 512],
                                    lhsT=m8_sb[:, 2 * pr : 2 * pr + 2, oc * P : (oc + 1) * P],
                                    rhs=f8_sb[:, 2 * pr : 2 * pr + 2,
                                              qb * 1024 + i * 512 : qb * 1024 + (i + 1) * 512],
                                    start=(jj == 0), stop=(jj == 3), perf_mode=DR,
                                )
                        ys0 = ypool.tile([P, 512], dt.bfloat16, tag="ys0")
                        nc.scalar.activation(
                            ys0[:], py[:, 0:512],
                            AF.Identity, bias=bias_fin[:, oc : oc + 1], scale=1.0,
                        )
                        nc.sync.dma_start(
                            yT_d[oc * P : (oc + 1) * P, qb * 1024 : qb * 1024 + 512],
                            ys0[:],
                        )
                        ys1 = ypool.tile([P, 512], dt.bfloat16, tag="ys1")
                        nc.vector.tensor_scalar_add(
                            ys1[:], py[:, 512:1024], bias_fin[:, oc : oc + 1]
                        )
                        nc.gpsimd.dma_start(
                            yT_d[oc * P : (oc + 1) * P,
                                 qb * 1024 + 512 : (qb + 1) * 1024],
                            ys1[:],
                        )
    _split_multi_waits(nc)
    return nc


def _get_program(debug=False):
    key = ("nc", debug)
    if key not in _CACHE:
        _CACHE[key] = _build_program(debug)
    return _CACHE[key]


def _f8(a, prescale):
    return np.clip(a * prescale, -240.0, 240.0).astype(_F8)


def _fm(a):
    """feature-major [P, DC] layout of a [D] vector: out[p, c] = a[c*P + p]"""
    return np.ascontiguousarray(a.astype(np.float32).reshape(DC, P).T)


def _prep_shared(Wq, bq, Wk, Wv, bv, Wo, bo):
    return {
        "wq8": _f8(np.ascontiguousarray(Wq.T), WS),
        "wk8": _f8(np.ascontiguousarray(Wk.T), WS),
        "wvb": np.ascontiguousarray(Wv.T).astype(_BF16),
        "wob": np.ascontiguousarray(Wo.T * 0.5).astype(_BF16),
        "bqs": _fm(SCALE * bq),
        "bos": _fm(bo),
        "tbv": _fm(float(T) * bv),
        "bvr": np.ascontiguousarray(bv.reshape(1, D)).astype(_BF16),
    }


def _run(in_maps, trace=False, debug=False, cores=None, **kw):
    from concourse.bass_utils import run_bass_kernel_spmd

    nc = _get_program(debug)
    if cores is None:
        cores = list(range(NCORES))
    return run_bass_kernel_spmd(nc, in_maps, cores, trace=trace, **kw)


def kernel(x, Wq, bq, Wk, Wv, bv, Wo, bo):
    x = np.asarray(x, dtype=np.float32)
    assert x.shape == (B, T, D), x.shape
    shared = _prep_shared(
        np.asarray(Wq, np.float32), np.asarray(bq, np.float32),
        np.asarray(Wk, np.float32), np.asarray(Wv, np.float32),
        np.asarray(bv, np.float32), np.asarray(Wo, np.float32),
        np.asarray(bo, np.float32),
    )
    in_maps = []
    for b in range(B):
        m = dict(shared)
        xb = x[b]
        m["xT8"] = _f8(np.ascontiguousarray(xb.T), XS)
        m["xt8"] = _f8(xb, XS)
        m["csx"] = _fm(xb.sum(axis=0))
        in_maps.append(m)

    res = _run(in_maps)
    out = np.empty((B, T, D), np.float32)
    for b in range(B):
        out[b] = res.results[b]["yT"].astype(np.float32).T
    return out


# revision 43
# speedup vs baseline: 2.9748x; 1.2240x over previous
"""Linear attention (silu+1 feature map) MultiHeadAttention kernel for 8x TRN2.

Sharding: data-parallel over batch (B=8 -> 1 batch element per NeuronCore).

fp8 DoubleRow formulation (all big GEMMs at fp8 2x rate, fp32 PSUM):

  stage 1 (stream 512-token tiles, feature-major x = xT8):
    fT[o,t] = silu(s*(WqT.T @ xT) + s*bq)        f = phi_q - 1, fp8 [P,DC,T]
    g[t,d]  = silu(s*(xT.T @ WkT))               g = phi_k - 1, fp8 [P,32,D]
    csg[d] += ones.T-row reductions of g          (DR matmuls, column form)
  A-GEMM (token-major x = xt8, 2 d-half passes, 8 PSUM banks):
    At[E,d] = sum_t x[t,E]*g[t,d]                 DR fp8
    Asb     = At/32 + csx[E]                      (csx = exact host colsum of x)
  kv assembly (bf16):
    kv_h[e,d] = Wv_h @ Asb[:,d_h] + bv_h (x) csg_h   (+ T*bv_h bias at repack)
    (identity: kv = phi_k^T v = Wv@(colsum_x + g^T x) + bv*(T + colsum_g))
  M stage:
    M_h[d,o] = kv_h.T @ (Wo_h/2)  -> m8 = fp8(2*pm) = fp8(M)
    colsum_M via rowsum(kv) hi/lo bf16 split @ Wo   -> phase-2 bias
  phase 2:
    yT[o,t] = m8.T @ f8 + (colsum_M + bo)         DR fp8; out bf16
    (identity: phi_q @ kv @ Wo = f @ M + colsum(M))

Host: fp8 casts (x*32 both layouts, W.T*1024), Wv.T/Wo.T*0.5 bf16, exact
colsum_x, T*bv, bias prep. Output bf16 -> fp32 on host.
"""

import numpy as np
import ml_dtypes

B, T, D = 8, 4096, 1024
H, DH = 16, 64
SCALE = float(DH ** -0.25)
NCORES = 8
P = 128
DC = D // P          # 8 feature chunks
TT = 512             # token tile (stage 1)
NTT = T // TT        # 8 token tiles
NSUB = TT // P       # 4 sub-tiles of 128 tokens
NG = T // P          # 32 token-major g/x subtiles
XS = 32.0            # x fp8 prescale
WS = 1024.0          # Wq/Wk fp8 prescale
QSCALE = SCALE / (XS * WS)
ASCALE = 1.0 / XS    # Asb descale

_BF16 = ml_dtypes.bfloat16
_F8 = ml_dtypes.float8_e4m3

_CACHE = {}


def _split_multi_waits(nc):
    """walrus in this container only encodes ONE sync-wait command per
    instruction. Hoist extra waits onto injected same-engine NOPs placed
    immediately before the instruction (program order on the engine queue
    makes this semantically identical)."""
    import concourse.mybir as mybir

    n_split = 0
    for fn in nc.m.functions:
        for bb in fn.blocks:
            new = []
            changed = False
            for inst in bb.instructions:
                si = inst.sync_info
                waits = list(si.on_wait) if si is not None else []
                if len(waits) > 1:
                    changed = True
                    for j, w in enumerate(waits[:-1]):
                        nop = mybir.InstNoOp(
                            name=f"{inst.name}-sw{j}", ins=[], outs=[]
                        )
                        nop.engine = inst.engine
                        nop.sync_info = mybir.SyncInfo(
                            on_wait=[w], on_update=[]
                        )
                        new.append(nop)
                        n_split += 1
                    inst.sync_info = mybir.SyncInfo(
                        on_wait=[waits[-1]], on_update=list(si.on_update)
                    )
                new.append(inst)
            if changed:
                bb.instructions = new
    return n_split


def _build_program(debug=False):
    import concourse.bass as bass
    import concourse.mybir as mybir
    from concourse.tile import TileContext, add_dep_helper

    dt = mybir.dt
    AF = mybir.ActivationFunctionType
    DR = mybir.MatmulPerfMode.DoubleRow
    ALU = mybir.AluOpType

    nc = bass.Bass()

    # all inputs host-pre-tiled to SBUF layout: every DMA is 128 descriptors
    # of >=4KB (descriptor generation on the trigger engines is the limiter)
    xT8_d = nc.dram_tensor("xT8", [NTT, P, DC * TT], dt.float8e4, kind="ExternalInput")
    xt8_d = nc.dram_tensor("xt8", [P, NG * D], dt.float8e4, kind="ExternalInput")
    wq8_d = nc.dram_tensor("wq8", [P, DC * D], dt.float8e4, kind="ExternalInput")
    wk8_d = nc.dram_tensor("wk8", [P, DC * D], dt.float8e4, kind="ExternalInput")
    wvb_d = nc.dram_tensor("wvb", [P, DC * D], dt.bfloat16, kind="ExternalInput")
    wob_d = nc.dram_tensor("wob", [P, DC * D], dt.bfloat16, kind="ExternalInput")
    bqs_d = nc.dram_tensor("bqs", [P, DC], dt.float32, kind="ExternalInput")
    bos_d = nc.dram_tensor("bos", [P, DC], dt.float32, kind="ExternalInput")
    csx_d = nc.dram_tensor("csx", [P, DC], dt.float32, kind="ExternalInput")
    tbv_d = nc.dram_tensor("tbv", [P, DC], dt.float32, kind="ExternalInput")
    bvr_d = nc.dram_tensor("bvr", [1, D], dt.bfloat16, kind="ExternalInput")
    yT_d = nc.dram_tensor("yT", [D, T], dt.bfloat16, kind="ExternalOutput")
    if debug:
        f_dump = nc.dram_tensor("f_dump", [P, DC, T], dt.float8e4, kind="ExternalOutput")
        g_dump = nc.dram_tensor("g_dump", [P, NG, D], dt.float8e4, kind="ExternalOutput")
        a_dump = nc.dram_tensor("a_dump", [P, DC, D], dt.bfloat16, kind="ExternalOutput")
        kv_dump = nc.dram_tensor("kv_dump", [P, DC, P], dt.bfloat16, kind="ExternalOutput")
        m_dump = nc.dram_tensor("m_dump", [P, DC, D], dt.float8e4, kind="ExternalOutput")
        csg_dump = nc.dram_tensor("csg_dump", [1, D], dt.bfloat16, kind="ExternalOutput")
        bias_dump = nc.dram_tensor("bias_dump", [P, DC], dt.float32, kind="ExternalOutput")

    with TileContext(nc) as tc:
        with (
            tc.tile_pool(name="weights", bufs=1) as wpool,
            tc.tile_pool(name="fstore", bufs=1) as fpool,
            tc.tile_pool(name="msb", bufs=1) as mpool,
        ):
            # pools that die before phase 2 (g, token-major x, x stream, Asb)
            # are scoped manually so phase 2 can reuse their SBUF for deep
            # y-output buffering
            _g_cm = tc.tile_pool(name="gstore", bufs=1)
            gpool = _g_cm.__enter__()
            _xk_cm = tc.tile_pool(name="xtok", bufs=1)
            xkpool = _xk_cm.__enter__()
            _x_cm = tc.tile_pool(name="xin", bufs=6)
            xpool = _x_cm.__enter__()
            _a_cm = tc.tile_pool(name="asb", bufs=1)
            apool = _a_cm.__enter__()
            # ---- weight / const preload ----
            wq_sb = wpool.tile([P, DC, D], dt.float8e4, tag="wq")
            wk_sb = wpool.tile([P, DC, D], dt.float8e4, tag="wk")
            wv_sb = wpool.tile([P, DC, D], dt.bfloat16, tag="wv")
            wo_sb = wpool.tile([P, DC, D], dt.bfloat16, tag="wo")
            bq_sb = wpool.tile([P, DC], dt.float32, tag="bq")
            bo_sb = wpool.tile([P, DC], dt.float32, tag="bo")
            csx_sb = wpool.tile([P, DC], dt.float32, tag="csx")
            tbv_sb = wpool.tile([P, DC], dt.float32, tag="tbv")
            bvr_sb = wpool.tile([1, D], dt.bfloat16, tag="bvr")

            zz = wpool.tile([1, 640], dt.bfloat16, tag="zz")
            nc.vector.memset(zz[:], 0.0)
            # weights on the sync queue; x tiles on gpsimd; token-major x on
            # the scalar queue — three queues run in parallel at startup.
            nc.sync.dma_start(wq_sb[:], wq8_d[:])
            xt_pre = []
            for half in range(2):
                xt0 = xpool.tile([P, DC, TT], dt.float8e4, tag="xt", name=f"xtpre{half}")
                nc.gpsimd.dma_start(xt0[:], xT8_d[half])
                xt_pre.append(xt0)
            nc.sync.dma_start(bq_sb[:], bqs_d[:])
            nc.sync.dma_start(bo_sb[:], bos_d[:])
            nc.sync.dma_start(wk_sb[:], wk8_d[:])

            # token-major x for the A-GEMM (needed only after stage 1):
            # its DMAs are deferred into the pair loop so they don't hog the
            # (exclusive) DMA engines while stage-1 weights/tiles load
            xtok_sb = xkpool.tile([P, NG, D], dt.float8e4, tag="xtok")
            nc.sync.dma_start(csx_sb[:], csx_d[:])
            nc.sync.dma_start(tbv_sb[:], tbv_d[:])
            nc.sync.dma_start(bvr_sb[:], bvr_d[:])

            f8_sb = fpool.tile([P, DC, T], dt.float8e4, tag="f8")
            g_sb = gpool.tile([P, NG, D], dt.float8e4, tag="g8")
            asb = apool.tile([P, DC, D], dt.bfloat16, tag="asb")
            m8_sb = mpool.tile([P, DC, D], dt.float8e4, tag="m8")
            kv_chunks = []
            for c in range(DC):
                kvc = mpool.tile([P, P], dt.bfloat16, tag=f"kvsb{c}", name=f"kvsb{c}")
                kv_chunks.append(kvc)
            csg_row = mpool.tile([1, D], dt.bfloat16, tag="csgrow")
            rs_f32 = mpool.tile([P, DC], dt.float32, tag="rsf32")
            rs_hi = mpool.tile([P, DC], dt.bfloat16, tag="rshi")
            rs_lo = mpool.tile([P, DC], dt.bfloat16, tag="rslo")
            bias_fin = mpool.tile([P, DC], dt.float32, tag="biasfin")

            for c in range(DC):
                nc.vector.memset(kv_chunks[c][:], 0.0)

            # ================= stage 1: projections =================
            with (
                tc.tile_pool(name="ps_q", bufs=2, space="PSUM") as pq_pool,
                tc.tile_pool(name="ps_k", bufs=2, space="PSUM") as pk_pool,
            ):
                # keep PE p-state warm under the startup DMA shadow
                warm = pk_pool.tile([P, D], dt.float32, tag="pk", name="warm")
                for w in range(36):
                    nc.tensor.matmul(
                        warm[:, 0:128], lhsT=zz[:1, :P], rhs=zz[:1, P : P + 128],
                        start=True, stop=True, skip_group_check=True,
                    )

                first_act = {}

                def _q_section(pair, xts):
                    # f[o-feat, token] = silu(SCALE*q + SCALE*bq), fp8 store
                    for oc in range(DC):
                        ps = pq_pool.tile([P, 2 * TT], dt.float32, tag="psq")
                        for j in range(4):
                            for half in range(2):
                                nc.tensor.matmul(
                                    ps[:, half * TT : (half + 1) * TT],
                                    lhsT=wq_sb[:, 2 * j : 2 * j + 2, oc * P : (oc + 1) * P],
                                    rhs=xts[half][:, 2 * j : 2 * j + 2, :],
                                    start=(j == 0), stop=(j == 3), perf_mode=DR,
                                )
                        a = nc.scalar.activation(
                            f8_sb[:, oc, pair * 1024 : (pair + 1) * 1024], ps[:],
                            AF.Silu, bias=bq_sb[:, oc : oc + 1], scale=QSCALE,
                        )
                        if oc == 0:
                            first_act[pair] = a

                last_mm = [None]

                def _k_section(pair, xts):
                    # g[token, d-feat] = silu(SCALE*k), fp8 store token-major
                    for half in range(2):
                        tt = pair * 2 + half
                        xt = xts[half]
                        for sub in range(NSUB):
                            gs = tt * NSUB + sub     # global 128-token subtile
                            pk = pk_pool.tile([P, D], dt.float32, tag="pk")
                            for j in range(4):
                                for n in range(2):
                                    last_mm[0] = nc.tensor.matmul(
                                        pk[:, n * 512 : (n + 1) * 512],
                                        lhsT=xt[:, 2 * j : 2 * j + 2, sub * P : (sub + 1) * P],
                                        rhs=wk_sb[:, 2 * j : 2 * j + 2, n * 512 : (n + 1) * 512],
                                        start=(j == 0), stop=(j == 3), perf_mode=DR,
                                    )
                            nc.scalar.activation(
                                g_sb[:, gs, :], pk[:], AF.Silu, scale=QSCALE,
                            )

                # dim-1 step must be a multiple of 16B for DoubleRow APs
                ones8 = wpool.tile([P, 2, 16], dt.float8e4, tag="ones8")
                nc.vector.memset(ones8[:], 1.0)

                xt_tiles = {0: xt_pre}

                def _fetch(pairq):
                    xts = []
                    for half in range(2):
                        tt = pairq * 2 + half
                        xt = xpool.tile([P, DC, TT], dt.float8e4, tag="xt")
                        nc.gpsimd.dma_start(xt[:], xT8_d[tt])
                        xts.append(xt)
                    xt_tiles[pairq] = xts

                _fetch(1)
                for pair in range(NTT // 2):
                    if pair + 2 < NTT // 2:
                        _fetch(pair + 2)
                    xts = xt_tiles.pop(pair)
                    _q_section(pair, xts)
                    _k_section(pair, xts)
                    # deferred bulk DMAs, gated on this pair's progress and
                    # sliced <=1MB so x tiles never wait long on the shared
                    # DMA engines
                    anchor = first_act[pair]
                    for ch in (2 * pair, 2 * pair + 1):
                        dma = nc.sync.dma_start(
                            xtok_sb[:, ch * 4 : (ch + 1) * 4, :],
                            xt8_d[:, ch * 4 * D : (ch + 1) * 4 * D],
                        )
                        add_dep_helper(dma.ins, anchor.ins, sync=True,
                                       reason="defer xtok behind stage 1")
                    wsb, wd = (wv_sb, wvb_d) if pair < 2 else (wo_sb, wob_d)
                    hh = pair % 2
                    dma = nc.scalar.dma_start(
                        wsb[:, hh * 4 : (hh + 1) * 4, :],
                        wd[:, hh * 4 * D : (hh + 1) * 4 * D],
                    )
                    add_dep_helper(dma.ins, anchor.ins, sync=True,
                                   reason="defer wv/wo")


            # ================= A-GEMM: At[E,d] = x^T g =================
            with tc.tile_pool(name="ps_a", bufs=1, space="PSUM") as pa_pool:
                pa = [
                    pa_pool.tile([P, 512], dt.float32, tag=f"pa{e}", name=f"pa{e}")
                    for e in range(DC)
                ]
                for dh in range(2):
                    for e in range(DC):
                        for s in range(NG // 2):
                            mm = nc.tensor.matmul(
                                pa[e][:],
                                lhsT=xtok_sb[:, 2 * s : 2 * s + 2, e * P : (e + 1) * P],
                                rhs=g_sb[:, 2 * s : 2 * s + 2, dh * 512 : (dh + 1) * 512],
                                start=(s == 0), stop=(s == NG // 2 - 1),
                                perf_mode=DR,
                            )
                            if dh == 0 and s == 0 and last_mm[0] is not None:
                                add_dep_helper(
                                    mm.ins, last_mm[0].ins, sync=False,
                                    reason="A-GEMM after stage 1",
                                )
                        # Asb = At/32 + csx[E]  (exact colsum_x folded in);
                        # drains alternate ACT/DVE and pipeline behind the
                        # next chunk's matmuls
                        if e % 2 == 0:
                            nc.scalar.activation(
                                asb[:, e, dh * 512 : (dh + 1) * 512], pa[e][:],
                                AF.Identity, bias=csx_sb[:, e : e + 1], scale=ASCALE,
                            )
                        else:
                            nc.vector.tensor_scalar(
                                out=asb[:, e, dh * 512 : (dh + 1) * 512],
                                in0=pa[e][:],
                                scalar1=ASCALE,
                                scalar2=csx_sb[:, e : e + 1],
                                op0=mybir.AluOpType.mult,
                                op1=mybir.AluOpType.add,
                            )

            # ============ kv assembly + M stage + phase-2 bias ============
            with (
                tc.tile_pool(name="ps_kv", bufs=1, space="PSUM") as pkv_pool,
                tc.tile_pool(name="ps_m", bufs=2, space="PSUM") as pm_pool,
                tc.tile_pool(name="ps_bias", bufs=1, space="PSUM") as pb_pool,
                tc.tile_pool(name="ps_csg", bufs=1, space="PSUM") as pcsg_pool,
            ):
                kv_ps = pkv_pool.tile([P, 512], dt.float32, tag="kvacc")
                nc.tensor.matmul(
                    kv_ps[:], lhsT=zz[:1, :P], rhs=zz[:1, P : P + 512],
                    start=True, stop=True, skip_group_check=True,
                )
                # csg[d] = colsum of g, directly in row form (ones-lhsT DR)
                for dh in range(2):
                    csg_ps = pcsg_pool.tile([1, 512], dt.float32, tag=f"csg{dh}",
                                            name=f"csg{dh}")
                    for s in range(NG // 2):
                        nc.tensor.matmul(
                            csg_ps[:],
                            lhsT=ones8[:, 0:2, 0:1],
                            rhs=g_sb[:, 2 * s : 2 * s + 2, dh * 512 : (dh + 1) * 512],
                            start=(s == 0), stop=(s == NG // 2 - 1), perf_mode=DR,
                        )
                    nc.vector.tensor_copy(
                        out=csg_row[0:1, dh * 512 : (dh + 1) * 512], in_=csg_ps[:]
                    )

                # kv_h[e,d] = sum_E Wv[e,E] * Asb[E,d]
                for e in range(DC):
                    for h in range(H):
                        r0 = (h % 2) * 64
                        c0 = (h // 2) * 64
                        nc.tensor.matmul(
                            kv_ps[r0 : r0 + 64, c0 : c0 + 64],
                            lhsT=wv_sb[:, e, h * 64 : (h + 1) * 64],
                            rhs=asb[:, e, h * 64 : (h + 1) * 64],
                            start=False, stop=False, skip_group_check=True,
                        )
                # + bv (x) csg  (rank-1 per head)
                for h in range(H):
                    r0 = (h % 2) * 64
                    c0 = (h // 2) * 64
                    nc.tensor.matmul(
                        kv_ps[r0 : r0 + 64, c0 : c0 + 64],
                        lhsT=bvr_sb[0:1, h * 64 : (h + 1) * 64],
                        rhs=csg_row[0:1, h * 64 : (h + 1) * 64],
                        start=False, stop=(h == H - 1), skip_group_check=True,
                    )
                # repack to block-diag chunks (adding T*bv[e] bias), then
                # immediately M(c) and its m8 drain so PE/ACT stay busy;
                # rowsum/bias matmuls follow
                for c in range(DC):
                    for r0 in (0, 64):
                        nc.scalar.activation(
                            kv_chunks[c][r0 : r0 + 64, r0 : r0 + 64],
                            kv_ps[r0 : r0 + 64, c * 64 : (c + 1) * 64],
                            AF.Identity, bias=tbv_sb[r0 : r0 + 64, c : c + 1],
                            scale=1.0,
                            accum_out=rs_f32[r0 : r0 + 64, c : c + 1],
                        )
                    pm = pm_pool.tile([P, D], dt.float32, tag="pm")
                    for n in range(2):
                        nc.tensor.matmul(
                            pm[:, n * 512 : (n + 1) * 512],
                            lhsT=kv_chunks[c][:],
                            rhs=wo_sb[:, c, n * 512 : (n + 1) * 512],
                            start=True, stop=True,
                        )
                    nc.scalar.mul(m8_sb[:, c, 0:512], pm[:, 0:512], 2.0)
                    nc.vector.tensor_scalar_mul(
                        m8_sb[:, c, 512:1024], pm[:, 512:1024], 2.0
                    )

                nc.gpsimd.tensor_copy(out=rs_hi[:], in_=rs_f32[:])
                nc.gpsimd.tensor_tensor(
                    rs_lo[:], rs_f32[:], rs_hi[:], mybir.AluOpType.subtract,
                )

                bias_ps = pb_pool.tile([P, DC], dt.float32, tag="biasps")
                for oc in range(DC):
                    for ci in range(2 * DC):
                        c, rs = ci // 2, (rs_hi if ci % 2 == 0 else rs_lo)
                        nc.tensor.matmul(
                            bias_ps[:, oc : oc + 1],
                            lhsT=wo_sb[:, c, oc * P : (oc + 1) * P],
                            rhs=rs[:, c : c + 1],
                            start=(ci == 0), stop=(ci == 2 * DC - 1),
                        )
                # bias_fin = 2*bias_ps + bo   (wo was halved on host)
                nc.vector.tensor_scalar_mul(bias_fin[:], bias_ps[:], 2.0)
                nc.vector.tensor_tensor(
                    bias_fin[:], bias_fin[:], bo_sb[:], mybir.AluOpType.add,
                )


            if debug:
                nc.sync.dma_start(f_dump[:], f8_sb[:])
                nc.sync.dma_start(g_dump[:], g_sb[:])
                nc.sync.dma_start(a_dump[:], asb[:])
                for c in range(DC):
                    nc.sync.dma_start(kv_dump[:, c, :], kv_chunks[c][:])
                nc.sync.dma_start(m_dump[:], m8_sb[:])
                nc.sync.dma_start(csg_dump[:], csg_row[:])
                nc.sync.dma_start(bias_dump[:], bias_fin[:])

            # ================= phase 2: yT = m8.T @ f8 + bias =================
            _a_cm.__exit__(None, None, None)
            _x_cm.__exit__(None, None, None)
            _xk_cm.__exit__(None, None, None)
            _g_cm.__exit__(None, None, None)
            _y_cm = tc.tile_pool(name="yout", bufs=10)
            ypool = _y_cm.__enter__()
            with tc.tile_pool(name="ps_y", bufs=4, space="PSUM") as py_pool:
                for oc in range(DC):
                    for qb in range(4):
                        last = oc == DC - 1 and qb >= 2
                        if last:
                            # final block: two independent [128,512] pieces so
                            # the tail is one small drain chain
                            for i in range(2):
                                pyf = py_pool.tile([P, 512], dt.float32, tag="py")
                                for jj in range(4):
                                    pr = (oc + qb + jj) % 4
                                    nc.tensor.matmul(
                                        pyf[:],
                                        lhsT=m8_sb[:, 2 * pr : 2 * pr + 2, oc * P : (oc + 1) * P],
                                        rhs=f8_sb[:, 2 * pr : 2 * pr + 2,
                                                  qb * 1024 + i * 512 : qb * 1024 + (i + 1) * 512],
                                        start=(jj == 0), stop=(jj == 3), perf_mode=DR,
                                    )
                                ysf = ypool.tile([P, 512], dt.bfloat16, tag="ys")
                                if i == 0:
                                    nc.scalar.activation(
                                        ysf[:], pyf[:],
                                        AF.Identity, bias=bias_fin[:, oc : oc + 1], scale=1.0,
                                    )
                                    nc.sync.dma_start(
                                        yT_d[oc * P : (oc + 1) * P,
                                             qb * 1024 : qb * 1024 + 512],
                                        ysf[:],
                                    )
                                else:
                                    nc.vector.tensor_scalar_add(
                                        ysf[:], pyf[:], bias_fin[:, oc : oc + 1]
                                    )
                                    nc.scalar.dma_start(
                                        yT_d[oc * P : (oc + 1) * P,
                                             qb * 1024 + 512 : (qb + 1) * 1024],
                                        ysf[:],
                                    )
                            continue
                        py = py_pool.tile([P, 1024], dt.float32, tag="py")
                        for jj in range(4):
                            pr = (oc + qb + jj) % 4
                            for i in range(2):
                                nc.tensor.matmul(
                                    py[:, i * 512 : (i + 1) * 512],
                                    lhsT=m8_sb[:, 2 * pr : 2 * pr + 2, oc * P : (oc + 1) * P],
                                    rhs=f8_sb[:, 2 * pr : 2 * pr + 2,
                                              qb * 1024 + i * 512 : qb * 1024 + (i + 1) * 512],
                                    start=(jj == 0), stop=(jj == 3), perf_mode=DR,
                                )
                        ys = ypool.tile([P, 1024], dt.bfloat16, tag="ys")
                        nc.scalar.activation(
                            ys[:, 0:512], py[:, 0:512],
                            AF.Identity, bias=bias_fin[:, oc : oc + 1], scale=1.0,
                        )
                        nc.vector.tensor_scalar_add(
                            ys[:, 512:1024], py[:, 512:1024], bias_fin[:, oc : oc + 1]
                        )
                        q_eng = nc.sync if (oc * 4 + qb) % 2 == 0 else nc.gpsimd
                        q_eng.dma_start(
                            yT_d[oc * P : (oc + 1) * P, qb * 1024 : (qb + 1) * 1024],
                            ys[:],
                        )
            _y_cm.__exit__(None, None, None)
    _split_multi_waits(nc)
    return nc


def _get_program(debug=False):
    key = ("nc", debug)
    if key not in _CACHE:
        _CACHE[key] = _build_program(debug)
    return _CACHE[key]


def _f8(a, prescale):
    return np.clip(a * prescale, -240.0, 240.0).astype(_F8)


def _fm(a):
    """feature-major [P, DC] layout of a [D] vector: out[p, c] = a[c*P + p]"""
    return np.ascontiguousarray(a.astype(np.float32).reshape(DC, P).T)


def _wtile(wt):
    """[D_in, D_out] -> SBUF layout [P, DC*D]: row p holds (chunk, out)."""
    return np.ascontiguousarray(
        wt.reshape(DC, P, D).transpose(1, 0, 2).reshape(P, DC * D)
    )


def _prep_shared(Wq, bq, Wk, Wv, bv, Wo, bo):
    return {
        "wq8": _f8(_wtile(np.ascontiguousarray(Wq.T)), WS),
        "wk8": _f8(_wtile(np.ascontiguousarray(Wk.T)), WS),
        "wvb": _wtile(np.ascontiguousarray(Wv.T)).astype(_BF16),
        "wob": _wtile(np.ascontiguousarray(Wo.T * 0.5)).astype(_BF16),
        "bqs": _fm(SCALE * bq),
        "bos": _fm(bo),
        "tbv": _fm(float(T) * bv),
        "bvr": np.ascontiguousarray(bv.reshape(1, D)).astype(_BF16),
    }


def _run(in_maps, trace=False, debug=False, cores=None, **kw):
    from concourse.bass_utils import run_bass_kernel_spmd

    nc = _get_program(debug)
    if cores is None:
        cores = list(range(NCORES))
    return run_bass_kernel_spmd(nc, in_maps, cores, trace=trace, **kw)


def kernel(x, Wq, bq, Wk, Wv, bv, Wo, bo):
    x = np.asarray(x, dtype=np.float32)
    assert x.shape == (B, T, D), x.shape
    shared = _prep_shared(
        np.asarray(Wq, np.float32), np.asarray(bq, np.float32),
        np.asarray(Wk, np.float32), np.asarray(Wv, np.float32),
        np.asarray(bv, np.float32), np.asarray(Wo, np.float32),
        np.asarray(bo, np.float32),
    )
    in_maps = []
    for b in range(B):
        m = dict(shared)
        xb = x[b]
        xbT = np.ascontiguousarray(xb.T)
        m["xT8"] = _f8(
            xbT.reshape(DC, P, NTT, TT).transpose(2, 1, 0, 3).reshape(NTT, P, DC * TT),
            XS,
        )
        m["xt8"] = _f8(
            xb.reshape(NG, P, D).transpose(1, 0, 2).reshape(P, NG * D), XS
        )
        m["csx"] = _fm(xb.sum(axis=0))
        in_maps.append(m)

    res = _run(in_maps)
    out = np.empty((B, T, D), np.float32)
    for b in range(B):
        out[b] = res.results[b]["yT"].astype(np.float32).T
    return out


# revision 46
# speedup vs baseline: 3.0177x; 1.0144x over previous
"""Linear attention (silu+1 feature map) MultiHeadAttention kernel for 8x TRN2.

Sharding: data-parallel over batch (B=8 -> 1 batch element per NeuronCore).

fp8 DoubleRow formulation (all big GEMMs at fp8 2x rate, fp32 PSUM):

  stage 1 (stream 512-token tiles, feature-major x = xT8):
    fT[o,t] = silu(s*(WqT.T @ xT) + s*bq)        f = phi_q - 1, fp8 [P,DC,T]
    g[t,d]  = silu(s*(xT.T @ WkT))               g = phi_k - 1, fp8 [P,32,D]
    csg[d] += ones.T-row reductions of g          (DR matmuls, column form)
  A-GEMM (token-major x = xt8, 2 d-half passes, 8 PSUM banks):
    At[E,d] = sum_t x[t,E]*g[t,d]                 DR fp8
    Asb     = At/32 + csx[E]                      (csx = exact host colsum of x)
  kv assembly (bf16):
    kv_h[e,d] = Wv_h @ Asb[:,d_h] + bv_h (x) csg_h   (+ T*bv_h bias at repack)
    (identity: kv = phi_k^T v = Wv@(colsum_x + g^T x) + bv*(T + colsum_g))
  M stage:
    M_h[d,o] = kv_h.T @ (Wo_h/2)  -> m8 = fp8(2*pm) = fp8(M)
    colsum_M via rowsum(kv) hi/lo bf16 split @ Wo   -> phase-2 bias
  phase 2:
    yT[o,t] = m8.T @ f8 + (colsum_M + bo)         DR fp8; out bf16
    (identity: phi_q @ kv @ Wo = f @ M + colsum(M))

Host: fp8 casts (x*32 both layouts, W.T*1024), Wv.T/Wo.T*0.5 bf16, exact
colsum_x, T*bv, bias prep. Output bf16 -> fp32 on host.
"""

import numpy as np
import ml_dtypes

B, T, D = 8, 4096, 1024
H, DH = 16, 64
SCALE = float(DH ** -0.25)
NCORES = 8
P = 128
DC = D // P          # 8 feature chunks
TT = 512             # token tile (stage 1)
NTT = T // TT        # 8 token tiles
NSUB = TT // P       # 4 sub-tiles of 128 tokens
NG = T // P          # 32 token-major g/x subtiles
XS = 32.0            # x fp8 prescale
WS = 1024.0          # Wq/Wk fp8 prescale
QSCALE = SCALE / (XS * WS)
ASCALE = 1.0 / XS    # Asb descale

_BF16 = ml_dtypes.bfloat16
_F8 = ml_dtypes.float8_e4m3

_CACHE = {}


def _split_multi_waits(nc):
    """walrus in this container only encodes ONE sync-wait command per
    instruction. Hoist extra waits onto injected same-engine NOPs placed
    immediately before the instruction (program order on the engine queue
    makes this semantically identical)."""
    import concourse.mybir as mybir

    n_split = 0
    for fn in nc.m.functions:
        for bb in fn.blocks:
            new = []
            changed = False
            for inst in bb.instructions:
                si = inst.sync_info
                waits = list(si.on_wait) if si is not None else []
                if len(waits) > 1:
                    changed = True
                    for j, w in enumerate(waits[:-1]):
                        nop = mybir.InstNoOp(
                            name=f"{inst.name}-sw{j}", ins=[], outs=[]
                        )
                        nop.engine = inst.engine
                        nop.sync_info = mybir.SyncInfo(
                            on_wait=[w], on_update=[]
                        )
                        new.append(nop)
                        n_split += 1
                    inst.sync_info = mybir.SyncInfo(
                        on_wait=[waits[-1]], on_update=list(si.on_update)
                    )
                new.append(inst)
            if changed:
                bb.instructions = new
    return n_split


def _build_program(debug=False):
    import concourse.bass as bass
    import concourse.mybir as mybir
    from concourse.tile import TileContext, add_dep_helper

    dt = mybir.dt
    AF = mybir.ActivationFunctionType
    DR = mybir.MatmulPerfMode.DoubleRow
    ALU = mybir.AluOpType

    nc = bass.Bass()

    # all inputs host-pre-tiled to SBUF layout: every DMA is 128 descriptors
    # of >=4KB (descriptor generation on the trigger engines is the limiter)
    xT8_d = nc.dram_tensor("xT8", [NTT, P, DC * TT], dt.float8e4, kind="ExternalInput")
    xt8_d = nc.dram_tensor("xt8", [P, NG * D], dt.float8e4, kind="ExternalInput")
    wq8_d = nc.dram_tensor("wq8", [P, DC * D], dt.float8e4, kind="ExternalInput")
    wk8_d = nc.dram_tensor("wk8", [P, DC * D], dt.float8e4, kind="ExternalInput")
    wvb_d = nc.dram_tensor("wvb", [P, DC * D], dt.bfloat16, kind="ExternalInput")
    wob_d = nc.dram_tensor("wob", [P, DC * D], dt.bfloat16, kind="ExternalInput")
    bqs_d = nc.dram_tensor("bqs", [P, DC], dt.float32, kind="ExternalInput")
    bos_d = nc.dram_tensor("bos", [P, DC], dt.float32, kind="ExternalInput")
    csx_d = nc.dram_tensor("csx", [P, DC], dt.float32, kind="ExternalInput")
    tbv_d = nc.dram_tensor("tbv", [P, DC], dt.float32, kind="ExternalInput")
    bvr_d = nc.dram_tensor("bvr", [1, D], dt.bfloat16, kind="ExternalInput")
    yT_d = nc.dram_tensor("yT", [D, T], dt.bfloat16, kind="ExternalOutput")
    if debug:
        f_dump = nc.dram_tensor("f_dump", [P, DC, T], dt.float8e4, kind="ExternalOutput")
        g_dump = nc.dram_tensor("g_dump", [P, NG, D], dt.float8e4, kind="ExternalOutput")
        a_dump = nc.dram_tensor("a_dump", [P, DC, D], dt.bfloat16, kind="ExternalOutput")
        kv_dump = nc.dram_tensor("kv_dump", [P, DC, P], dt.bfloat16, kind="ExternalOutput")
        m_dump = nc.dram_tensor("m_dump", [P, DC, D], dt.float8e4, kind="ExternalOutput")
        csg_dump = nc.dram_tensor("csg_dump", [1, D], dt.bfloat16, kind="ExternalOutput")
        bias_dump = nc.dram_tensor("bias_dump", [P, DC], dt.float32, kind="ExternalOutput")

    with TileContext(nc) as tc:
        with (
            tc.tile_pool(name="weights", bufs=1) as wpool,
            tc.tile_pool(name="fstore", bufs=1) as fpool,
            tc.tile_pool(name="msb", bufs=1) as mpool,
        ):
            # pools that die before phase 2 (g, token-major x, x stream, Asb)
            # are scoped manually so phase 2 can reuse their SBUF for deep
            # y-output buffering
            _g_cm = tc.tile_pool(name="gstore", bufs=1)
            gpool = _g_cm.__enter__()
            _xk_cm = tc.tile_pool(name="xtok", bufs=1)
            xkpool = _xk_cm.__enter__()
            _x_cm = tc.tile_pool(name="xin", bufs=6)
            xpool = _x_cm.__enter__()
            _a_cm = tc.tile_pool(name="asb", bufs=1)
            apool = _a_cm.__enter__()
            # ---- weight / const preload ----
            wq_sb = wpool.tile([P, DC, D], dt.float8e4, tag="wq")
            wk_sb = wpool.tile([P, DC, D], dt.float8e4, tag="wk")
            wv_sb = wpool.tile([P, DC, D], dt.bfloat16, tag="wv")
            wo_sb = wpool.tile([P, DC, D], dt.bfloat16, tag="wo")
            bq_sb = wpool.tile([P, DC], dt.float32, tag="bq")
            bo_sb = wpool.tile([P, DC], dt.float32, tag="bo")
            csx_sb = wpool.tile([P, DC], dt.float32, tag="csx")
            tbv_sb = wpool.tile([P, DC], dt.float32, tag="tbv")
            bvr_sb = wpool.tile([1, D], dt.bfloat16, tag="bvr")

            zz = wpool.tile([1, 640], dt.bfloat16, tag="zz")
            nc.vector.memset(zz[:], 0.0)
            # weights on the sync queue; x tiles on gpsimd; token-major x on
            # the scalar queue — three queues run in parallel at startup.
            nc.sync.dma_start(wq_sb[:], wq8_d[:])
            xt_pre = []
            for half in range(2):
                xt0 = xpool.tile([P, DC, TT], dt.float8e4, tag="xt", name=f"xtpre{half}")
                nc.gpsimd.dma_start(xt0[:], xT8_d[half])
                xt_pre.append(xt0)
            nc.sync.dma_start(bq_sb[:], bqs_d[:])
            nc.sync.dma_start(bo_sb[:], bos_d[:])
            nc.sync.dma_start(wk_sb[:], wk8_d[:])

            # token-major x for the A-GEMM (needed only after stage 1):
            # its DMAs are deferred into the pair loop so they don't hog the
            # (exclusive) DMA engines while stage-1 weights/tiles load
            xtok_sb = xkpool.tile([P, NG, D], dt.float8e4, tag="xtok")
            nc.sync.dma_start(csx_sb[:], csx_d[:])
            nc.sync.dma_start(tbv_sb[:], tbv_d[:])
            nc.sync.dma_start(bvr_sb[:], bvr_d[:])

            f8_sb = fpool.tile([P, DC, T], dt.float8e4, tag="f8")
            g_sb = gpool.tile([P, NG, D], dt.float8e4, tag="g8")
            asb = apool.tile([P, DC, D], dt.bfloat16, tag="asb")
            m8_sb = mpool.tile([P, DC, D], dt.float8e4, tag="m8")
            kv_chunks = []
            for c in range(DC):
                kvc = mpool.tile([P, P], dt.bfloat16, tag=f"kvsb{c}", name=f"kvsb{c}")
                kv_chunks.append(kvc)
            csg_row = mpool.tile([1, D], dt.bfloat16, tag="csgrow")
            rs_f32 = mpool.tile([P, DC], dt.float32, tag="rsf32")
            rs_hi = mpool.tile([P, DC], dt.bfloat16, tag="rshi")
            rs_lo = mpool.tile([P, DC], dt.bfloat16, tag="rslo")
            bias_fin = mpool.tile([P, DC], dt.float32, tag="biasfin")

            for c in range(DC):
                nc.vector.memset(kv_chunks[c][:], 0.0)

            # ================= stage 1: projections =================
            with (
                tc.tile_pool(name="ps_q", bufs=2, space="PSUM") as pq_pool,
                tc.tile_pool(name="ps_k", bufs=2, space="PSUM") as pk_pool,
            ):
                # keep PE p-state warm under the startup DMA shadow
                warm = pk_pool.tile([P, D], dt.float32, tag="pk", name="warm")
                for w in range(36):
                    nc.tensor.matmul(
                        warm[:, 0:128], lhsT=zz[:1, :P], rhs=zz[:1, P : P + 128],
                        start=True, stop=True, skip_group_check=True,
                    )

                first_act = {}

                def _q_section(pair, xts):
                    # f[o-feat, token] = silu(SCALE*q + SCALE*bq), fp8 store
                    for oc in range(DC):
                        ps = pq_pool.tile([P, 2 * TT], dt.float32, tag="psq")
                        for j in range(4):
                            for half in range(2):
                                nc.tensor.matmul(
                                    ps[:, half * TT : (half + 1) * TT],
                                    lhsT=wq_sb[:, 2 * j : 2 * j + 2, oc * P : (oc + 1) * P],
                                    rhs=xts[half][:, 2 * j : 2 * j + 2, :],
                                    start=(j == 0), stop=(j == 3), perf_mode=DR,
                                )
                        a = nc.scalar.activation(
                            f8_sb[:, oc, pair * 1024 : (pair + 1) * 1024], ps[:],
                            AF.Silu, bias=bq_sb[:, oc : oc + 1], scale=QSCALE,
                        )
                        if oc == 0:
                            first_act[pair] = a

                last_mm = [None]

                def _k_section(pair, xts):
                    # g[token, d-feat] = silu(SCALE*k), fp8 store token-major
                    for half in range(2):
                        tt = pair * 2 + half
                        xt = xts[half]
                        for sub in range(NSUB):
                            gs = tt * NSUB + sub     # global 128-token subtile
                            pk = pk_pool.tile([P, D], dt.float32, tag="pk")
                            for j in range(4):
                                for n in range(2):
                                    last_mm[0] = nc.tensor.matmul(
                                        pk[:, n * 512 : (n + 1) * 512],
                                        lhsT=xt[:, 2 * j : 2 * j + 2, sub * P : (sub + 1) * P],
                                        rhs=wk_sb[:, 2 * j : 2 * j + 2, n * 512 : (n + 1) * 512],
                                        start=(j == 0), stop=(j == 3), perf_mode=DR,
                                    )
                            nc.scalar.activation(
                                g_sb[:, gs, :], pk[:], AF.Silu, scale=QSCALE,
                            )

                # dim-1 step must be a multiple of 16B for DoubleRow APs
                ones8 = wpool.tile([P, 2, 16], dt.float8e4, tag="ones8")
                nc.vector.memset(ones8[:], 1.0)

                xt_tiles = {0: xt_pre}

                def _fetch(pairq):
                    xts = []
                    for half in range(2):
                        tt = pairq * 2 + half
                        xt = xpool.tile([P, DC, TT], dt.float8e4, tag="xt")
                        nc.gpsimd.dma_start(xt[:], xT8_d[tt])
                        xts.append(xt)
                    xt_tiles[pairq] = xts

                _fetch(1)
                for pair in range(NTT // 2):
                    if pair + 2 < NTT // 2:
                        _fetch(pair + 2)
                    xts = xt_tiles.pop(pair)
                    _q_section(pair, xts)
                    _k_section(pair, xts)
                    # deferred bulk DMAs, gated on this pair's progress and
                    # sliced <=1MB so x tiles never wait long on the shared
                    # DMA engines
                    anchor = first_act[pair]
                    for ch in (2 * pair, 2 * pair + 1):
                        dma = nc.sync.dma_start(
                            xtok_sb[:, ch * 4 : (ch + 1) * 4, :],
                            xt8_d[:, ch * 4 * D : (ch + 1) * 4 * D],
                        )
                        add_dep_helper(dma.ins, anchor.ins, sync=True,
                                       reason="defer xtok behind stage 1")
                    wsb, wd = (wv_sb, wvb_d) if pair < 2 else (wo_sb, wob_d)
                    hh = pair % 2
                    dma = nc.scalar.dma_start(
                        wsb[:, hh * 4 : (hh + 1) * 4, :],
                        wd[:, hh * 4 * D : (hh + 1) * 4 * D],
                    )
                    add_dep_helper(dma.ins, anchor.ins, sync=True,
                                   reason="defer wv/wo")


            # ================= A-GEMM: At[E,d] = x^T g =================
            with tc.tile_pool(name="ps_a", bufs=1, space="PSUM") as pa_pool:
                pa = [
                    pa_pool.tile([P, 512], dt.float32, tag=f"pa{e}", name=f"pa{e}")
                    for e in range(DC)
                ]
                for dh in range(2):
                    for e in range(DC):
                        for s in range(NG // 2):
                            mm = nc.tensor.matmul(
                                pa[e][:],
                                lhsT=xtok_sb[:, 2 * s : 2 * s + 2, e * P : (e + 1) * P],
                                rhs=g_sb[:, 2 * s : 2 * s + 2, dh * 512 : (dh + 1) * 512],
                                start=(s == 0), stop=(s == NG // 2 - 1),
                                perf_mode=DR,
                            )
                            if dh == 0 and s == 0 and last_mm[0] is not None:
                                add_dep_helper(
                                    mm.ins, last_mm[0].ins, sync=False,
                                    reason="A-GEMM after stage 1",
                                )
                        # Asb = At/32 + csx[E]  (exact colsum_x folded in);
                        # drains alternate ACT/DVE and pipeline behind the
                        # next chunk's matmuls
                        if e % 2 == 0:
                            nc.scalar.activation(
                                asb[:, e, dh * 512 : (dh + 1) * 512], pa[e][:],
                                AF.Identity, bias=csx_sb[:, e : e + 1], scale=ASCALE,
                            )
                        else:
                            nc.vector.tensor_scalar(
                                out=asb[:, e, dh * 512 : (dh + 1) * 512],
                                in0=pa[e][:],
                                scalar1=ASCALE,
                                scalar2=csx_sb[:, e : e + 1],
                                op0=mybir.AluOpType.mult,
                                op1=mybir.AluOpType.add,
                            )

            # ============ kv assembly + M stage + phase-2 bias ============
            with (
                tc.tile_pool(name="ps_kv", bufs=1, space="PSUM") as pkv_pool,
                tc.tile_pool(name="ps_m", bufs=2, space="PSUM") as pm_pool,
                tc.tile_pool(name="ps_bias", bufs=1, space="PSUM") as pb_pool,
                tc.tile_pool(name="ps_csg", bufs=1, space="PSUM") as pcsg_pool,
            ):
                kv_ps = pkv_pool.tile([P, 512], dt.float32, tag="kvacc")
                nc.tensor.matmul(
                    kv_ps[:], lhsT=zz[:1, :P], rhs=zz[:1, P : P + 512],
                    start=True, stop=True, skip_group_check=True,
                )
                # csg[d] = colsum of g, directly in row form (ones-lhsT DR)
                for dh in range(2):
                    csg_ps = pcsg_pool.tile([1, 512], dt.float32, tag=f"csg{dh}",
                                            name=f"csg{dh}")
                    for s in range(NG // 2):
                        nc.tensor.matmul(
                            csg_ps[:],
                            lhsT=ones8[:, 0:2, 0:1],
                            rhs=g_sb[:, 2 * s : 2 * s + 2, dh * 512 : (dh + 1) * 512],
                            start=(s == 0), stop=(s == NG // 2 - 1), perf_mode=DR,
                        )
                    nc.vector.tensor_copy(
                        out=csg_row[0:1, dh * 512 : (dh + 1) * 512], in_=csg_ps[:]
                    )

                # kv_h[e,d] = sum_E Wv[e,E] * Asb[E,d]
                for e in range(DC):
                    for h in range(H):
                        r0 = (h % 2) * 64
                        c0 = (h // 2) * 64
                        nc.tensor.matmul(
                            kv_ps[r0 : r0 + 64, c0 : c0 + 64],
                            lhsT=wv_sb[:, e, h * 64 : (h + 1) * 64],
                            rhs=asb[:, e, h * 64 : (h + 1) * 64],
                            start=False, stop=False, skip_group_check=True,
                        )
                # + bv (x) csg  (rank-1 per head)
                for h in range(H):
                    r0 = (h % 2) * 64
                    c0 = (h // 2) * 64
                    nc.tensor.matmul(
                        kv_ps[r0 : r0 + 64, c0 : c0 + 64],
                        lhsT=bvr_sb[0:1, h * 64 : (h + 1) * 64],
                        rhs=csg_row[0:1, h * 64 : (h + 1) * 64],
                        start=False, stop=(h == H - 1), skip_group_check=True,
                    )
                # repack to block-diag chunks (adding T*bv[e] bias), then
                # immediately M(c) and its m8 drain so PE/ACT stay busy;
                # rowsum/bias matmuls follow
                for c in range(DC):
                    for r0 in (0, 64):
                        nc.scalar.activation(
                            kv_chunks[c][r0 : r0 + 64, r0 : r0 + 64],
                            kv_ps[r0 : r0 + 64, c * 64 : (c + 1) * 64],
                            AF.Identity, bias=tbv_sb[r0 : r0 + 64, c : c + 1],
                            scale=1.0,
                            accum_out=rs_f32[r0 : r0 + 64, c : c + 1],
                        )
                    pm = pm_pool.tile([P, D], dt.float32, tag="pm")
                    for n in range(2):
                        nc.tensor.matmul(
                            pm[:, n * 512 : (n + 1) * 512],
                            lhsT=kv_chunks[c][:],
                            rhs=wo_sb[:, c, n * 512 : (n + 1) * 512],
                            start=True, stop=True,
                        )
                    nc.vector.tensor_scalar_mul(m8_sb[:, c, :], pm[:], 2.0)

                nc.gpsimd.tensor_copy(out=rs_hi[:], in_=rs_f32[:])
                nc.gpsimd.tensor_tensor(
                    rs_lo[:], rs_f32[:], rs_hi[:], mybir.AluOpType.subtract,
                )

                bias_ps = pb_pool.tile([P, DC], dt.float32, tag="biasps")
                for oc in range(DC):
                    for ci in range(2 * DC):
                        c, rs = ci // 2, (rs_hi if ci % 2 == 0 else rs_lo)
                        nc.tensor.matmul(
                            bias_ps[:, oc : oc + 1],
                            lhsT=wo_sb[:, c, oc * P : (oc + 1) * P],
                            rhs=rs[:, c : c + 1],
                            start=(ci == 0), stop=(ci == 2 * DC - 1),
                        )
                # bias_fin = 2*bias_ps + bo   (wo was halved on host)
                nc.vector.tensor_scalar_mul(bias_fin[:], bias_ps[:], 2.0)
                nc.vector.tensor_tensor(
                    bias_fin[:], bias_fin[:], bo_sb[:], mybir.AluOpType.add,
                )


            if debug:
                nc.sync.dma_start(f_dump[:], f8_sb[:])
                nc.sync.dma_start(g_dump[:], g_sb[:])
                nc.sync.dma_start(a_dump[:], asb[:])
                for c in range(DC):
                    nc.sync.dma_start(kv_dump[:, c, :], kv_chunks[c][:])
                nc.sync.dma_start(m_dump[:], m8_sb[:])
                nc.sync.dma_start(csg_dump[:], csg_row[:])
                nc.sync.dma_start(bias_dump[:], bias_fin[:])

            # ================= phase 2: yT = m8.T @ f8 + bias =================
            _a_cm.__exit__(None, None, None)
            _x_cm.__exit__(None, None, None)
            _xk_cm.__exit__(None, None, None)
            _g_cm.__exit__(None, None, None)
            _y_cm = tc.tile_pool(name="yout", bufs=10)
            ypool = _y_cm.__enter__()
            with tc.tile_pool(name="ps_y", bufs=4, space="PSUM") as py_pool:
                for oc in range(DC):
                    for qb in range(4):
                        last = oc == DC - 1 and qb >= 2
                        if last:
                            # final block: two independent [128,512] pieces so
                            # the tail is one small drain chain
                            for i in range(2):
                                pyf = py_pool.tile([P, 512], dt.float32, tag="py")
                                for jj in range(4):
                                    pr = (oc + qb + jj) % 4
                                    nc.tensor.matmul(
                                        pyf[:],
                                        lhsT=m8_sb[:, 2 * pr : 2 * pr + 2, oc * P : (oc + 1) * P],
                                        rhs=f8_sb[:, 2 * pr : 2 * pr + 2,
                                                  qb * 1024 + i * 512 : qb * 1024 + (i + 1) * 512],
                                        start=(jj == 0), stop=(jj == 3), perf_mode=DR,
                                    )
                                ysf = ypool.tile([P, 512], dt.bfloat16, tag="ys")
                                if i == 0:
                                    nc.scalar.activation(
                                        ysf[:], pyf[:],
                                        AF.Identity, bias=bias_fin[:, oc : oc + 1], scale=1.0,
                                    )
                                    nc.sync.dma_start(
                                        yT_d[oc * P : (oc + 1) * P,
                                             qb * 1024 : qb * 1024 + 512],
                                        ysf[:],
                                    )
                                else:
                                    nc.vector.tensor_scalar_add(
                                        ysf[:], pyf[:], bias_fin[:, oc : oc + 1]
                                    )
                                    nc.scalar.dma_start(
                                        yT_d[oc * P : (oc + 1) * P,
                                             qb * 1024 + 512 : (qb + 1) * 1024],
                                        ysf[:],
                                    )
                            continue
                        py = py_pool.tile([P, 1024], dt.float32, tag="py")
                        for jj in range(4):
                            pr = (oc + qb + jj) % 4
                            for i in range(2):
                                nc.tensor.matmul(
                                    py[:, i * 512 : (i + 1) * 512],
                                    lhsT=m8_sb[:, 2 * pr : 2 * pr + 2, oc * P : (oc + 1) * P],
                                    rhs=f8_sb[:, 2 * pr : 2 * pr + 2,
                                              qb * 1024 + i * 512 : qb * 1024 + (i + 1) * 512],
                                    start=(jj == 0), stop=(jj == 3), perf_mode=DR,
                                )
                        ys = ypool.tile([P, 1024], dt.bfloat16, tag="ys")
                        nc.scalar.activation(
                            ys[:, 0:512], py[:, 0:512],
                            AF.Identity, bias=bias_fin[:, oc : oc + 1], scale=1.0,
                        )
                        nc.vector.tensor_scalar_add(
                            ys[:, 512:1024], py[:, 512:1024], bias_fin[:, oc : oc + 1]
                        )
                        q_eng = nc.sync if (oc * 4 + qb) % 2 == 0 else nc.gpsimd
                        q_eng.dma_start(
                            yT_d[oc * P : (oc + 1) * P, qb * 1024 : (qb + 1) * 1024],
                            ys[:],
                        )
            _y_cm.__exit__(None, None, None)
    _split_multi_waits(nc)
    return nc


def _get_program(debug=False):
    key = ("nc", debug)
    if key not in _CACHE:
        _CACHE[key] = _build_program(debug)
    return _CACHE[key]


def _f8(a, prescale):
    return np.clip(a * prescale, -240.0, 240.0).astype(_F8)


def _fm(a):
    """feature-major [P, DC] layout of a [D] vector: out[p, c] = a[c*P + p]"""
    return np.ascontiguousarray(a.astype(np.float32).reshape(DC, P).T)


def _wtile(wt):
    """[D_in, D_out] -> SBUF layout [P, DC*D]: row p holds (chunk, out)."""
    return np.ascontiguousarray(
        wt.reshape(DC, P, D).transpose(1, 0, 2).reshape(P, DC * D)
    )


def _prep_shared(Wq, bq, Wk, Wv, bv, Wo, bo):
    return {
        "wq8": _f8(_wtile(np.ascontiguousarray(Wq.T)), WS),
        "wk8": _f8(_wtile(np.ascontiguousarray(Wk.T)), WS),
        "wvb": _wtile(np.ascontiguousarray(Wv.T)).astype(_BF16),
        "wob": _wtile(np.ascontiguousarray(Wo.T * 0.5)).astype(_BF16),
        "bqs": _fm(SCALE * bq),
        "bos": _fm(bo),
        "tbv": _fm(float(T) * bv),
        "bvr": np.ascontiguousarray(bv.reshape(1, D)).astype(_BF16),
    }


def _run(in_maps, trace=False, debug=False, cores=None, **kw):
    from concourse.bass_utils import run_bass_kernel_spmd

    nc = _get_program(debug)
    if cores is None:
        cores = list(range(NCORES))
    return run_bass_kernel_spmd(nc, in_maps, cores, trace=trace, **kw)


def kernel(x, Wq, bq, Wk, Wv, bv, Wo, bo):
    x = np.asarray(x, dtype=np.float32)
    assert x.shape == (B, T, D), x.shape
    shared = _prep_shared(
        np.asarray(Wq, np.float32), np.asarray(bq, np.float32),
        np.asarray(Wk, np.float32), np.asarray(Wv, np.float32),
        np.asarray(bv, np.float32), np.asarray(Wo, np.float32),
        np.asarray(bo, np.float32),
    )
    in_maps = []
    for b in range(B):
        m = dict(shared)
        xb = x[b]
        xbT = np.ascontiguousarray(xb.T)
        m["xT8"] = _f8(
            xbT.reshape(DC, P, NTT, TT).transpose(2, 1, 0, 3).reshape(NTT, P, DC * TT),
            XS,
        )
        m["xt8"] = _f8(
            xb.reshape(NG, P, D).transpose(1, 0, 2).reshape(P, NG * D), XS
        )
        m["csx"] = _fm(xb.sum(axis=0))
        in_maps.append(m)

    res = _run(in_maps)
    out = np.empty((B, T, D), np.float32)
    for b in range(B):
        out[b] = res.results[b]["yT"].astype(np.float32).T
    return out
